# revision 79
# baseline (speedup 1.0000x reference)
"""Bass/Tile kernel for nn_MultiHeadAttention (B=2, S=2048, D=1024, H=16).

Sharding: 8 cores = 2 (batch) x 4 (head-chunks of 4 heads).
Each core computes, for its batch b and its 4 heads (2 pairs of 2):
  qpT/kpT = (x @ W{q,k} + b)^T  in [dout, token] fp16 layout
  vp      = v @ Wv + bv         in [token, dout] bf16 layout
  scoresT = kp @ qp^T           per head, [k, q] fp32 PSUM
  attnT   = exp(scoresT)        (softmax over q == free axis) -> bf16
  Z[k]    = sum_q attnT[k, q]   (ACT accum_out, fp32)
  outT    = sum_kb (vp[kb]/Z[kb]) PV matmuls, accumulated IN PSUM
            across all 16 k-blocks (pvt0/pvt1 resident banks)
  out_p   = hcT_p^T @ Wo_p      per-pair fp16 partials (host sums 8)

PSUM: sc tag bufs=2 (4 banks, shared by scores AND all projection
chunks) + pvt0/pvt1 resident accumulators (4 banks) = 8.
Schedule: pair-0 q/k proj head; v-proj + remaining projections trickle
through pair-0 attention; pair-0's O-projection trickles through
pair-1 attention; tail is only pair-1's O-projection.
"""

import sys

sys.path.insert(0, "/opt/trn_rl_repo")

from contextlib import ExitStack

import numpy as np
import ml_dtypes

import concourse.bass as bass
import concourse.mybir as mybir
import concourse.tile as tile
from concourse import bacc
from concourse.bass_utils import run_bass_kernel_spmd

BF16 = mybir.dt.bfloat16
F16 = mybir.dt.float16
F32 = mybir.dt.float32
AF = mybir.ActivationFunctionType
ALU = mybir.AluOpType

D = 1024
NK = 8  # k-tiles over D
DOUT = 256  # per-core head dims (4 heads)
NPAIR = 2  # pairs of heads (128 dout each)
HD = 64


def build_kernel(S=2048):
    NKB = S // 128  # k-token blocks
    NQH = S // 1024  # exp blocks of 1024 along q
    NTC = S // 512  # proj token chunks
    NTT = S // 128  # token tiles
    assert S % 1024 == 0

    nc = bacc.Bacc("TRN2", target_bir_lowering=False, debug=False)

    qT = nc.dram_tensor("qT", [D, S], F16, kind="ExternalInput")
    kT = nc.dram_tensor("kT", [D, S], F16, kind="ExternalInput")
    vT = nc.dram_tensor("vT", [D, S], BF16, kind="ExternalInput")
    wq = nc.dram_tensor("wq", [D, DOUT], F16, kind="ExternalInput")
    wk = nc.dram_tensor("wk", [D, DOUT], F16, kind="ExternalInput")
    wv = nc.dram_tensor("wv", [D, DOUT], BF16, kind="ExternalInput")
    wo = nc.dram_tensor("wo", [DOUT, D], F16, kind="ExternalInput")
    bq = nc.dram_tensor("bq", [NPAIR, 128, 1], F32, kind="ExternalInput")
    bk = nc.dram_tensor("bk", [NPAIR, 128, 1], F32, kind="ExternalInput")
    bv = nc.dram_tensor("bv", [DOUT], F32, kind="ExternalInput")
    out0 = nc.dram_tensor("out0", [S, D], F16, kind="ExternalOutput")
    out1 = nc.dram_tensor("out1", [S, D], F16, kind="ExternalOutput")

    # tiled DRAM views
    qTv = qT.ap().rearrange("(t p) s -> t p s", p=128)  # [8, 128, S]
    kTv = kT.ap().rearrange("(t p) s -> t p s", p=128)
    vTv = vT.ap().rearrange("(t p) s -> t p s", p=128)
    wqv = wq.ap().rearrange("(t p) m -> p t m", p=128)  # [128, 8, 256]
    wkv = wk.ap().rearrange("(t p) m -> p t m", p=128)
    wvv = wv.ap().rearrange("(t p) m -> p t m", p=128)
    wov = wo.ap().rearrange("(t p) m -> p t m", p=128)  # [128, 2, 1024]
    bqv = bq.ap().rearrange("a p o -> p a o")  # [128, 2, 1]
    bkv = bk.ap().rearrange("a p o -> p a o")
    outv = [
        out0.ap().rearrange("(t p) m -> t p m", p=128),  # [NTT, 128, 1024]
        out1.ap().rearrange("(t p) m -> t p m", p=128),
    ]

    bv_bcast_ap = bass.AP(tensor=bv.ap().tensor, offset=0, ap=[[0, 128], [1, DOUT]])

    with tile.TileContext(nc) as tc, ExitStack() as ctx:
        sb = ctx.enter_context(tc.tile_pool(name="sb", bufs=1))

        # ---- resident weight/bias loads ----
        wq_sb = sb.tile([128, NK, DOUT], F16, tag="wq")
        wk_sb = sb.tile([128, NK, DOUT], F16, tag="wk")
        wv_sb = sb.tile([128, NK, DOUT], BF16, tag="wv")
        wo_sb = sb.tile([128, NPAIR, D], F16, tag="wo")
        bq_sb = sb.tile([128, NPAIR, 1], F32, tag="bq")
        bk_sb = sb.tile([128, NPAIR, 1], F32, tag="bk")
        bv_sb = sb.tile([128, DOUT], F32, tag="bv")

        # ---- resident activations ----
        qT_sb = sb.tile([128, NK, S], F16, tag="qT")
        kT_sb = sb.tile([128, NK, S], F16, tag="kT")
        vT_sb = sb.tile([128, NK, S], BF16, tag="vT")

        # head loads, ordered by first use; big tensors split into
        # token-slices so compute starts on partial data (DMA engines
        # serialize in practice)
        nc.sync.dma_start(out=bq_sb[:], in_=bqv)
        nc.sync.dma_start(out=wq_sb[:], in_=wqv)
        s0 = slice(0, 512)
        for kk in range(NK):  # q tokens 0:1024 -> proj chunks 0,1
            nc.sync.dma_start(out=qT_sb[:, kk, 0:1024], in_=qTv[kk][:, 0:1024])
        # k-path weights aren't needed until the k-proj chunk (~11us in)
        nc.sync.dma_start(out=wk_sb[:], in_=wkv)
        nc.sync.dma_start(out=bk_sb[:], in_=bkv)
        for kk in range(NK):  # k tokens 0:512 -> k-proj chunk 0
            nc.sync.dma_start(out=kT_sb[:, kk, s0], in_=kTv[kk][:, s0])
        for kk in range(NK):  # q tokens 1024:2048 -> proj chunks 2,3
            nc.sync.dma_start(out=qT_sb[:, kk, 1024:2048], in_=qTv[kk][:, 1024:2048])
        # v-path weights are not needed until v-proj (~20us in)
        nc.sync.dma_start(out=wv_sb[:], in_=wvv)
        nc.sync.dma_start(out=bv_sb[:], in_=bv_bcast_ap)

        # ---- projection outputs (resident SBUF) ----
        qpT_sb = sb.tile([128, NPAIR, S], F16, tag="qpT")
        kpT_sb = sb.tile([128, NPAIR, S], F16, tag="kpT")
        vp_sb = sb.tile([128, NTT, DOUT], BF16, tag="vp")
        hcT_sb = sb.tile([128, NPAIR, S], F16, tag="hcT")

        # PSUM: sc bufs=2 (4 banks, scores + all projections) +
        # pvt0/pvt1 resident PV accumulators (4 banks) = 8
        psa = ctx.enter_context(tc.tile_pool(name="ps_all", bufs=1, space="PSUM"))
        asb = ctx.enter_context(tc.tile_pool(name="att_sb", bufs=1))
        osb = ctx.enter_context(tc.tile_pool(name="o_sb", bufs=1))

        # warm the exp table during head DMA (table load ~2.7us)
        warm = sb.tile([128, 1], F32, tag="warm")
        nc.scalar.activation(out=warm[:], in_=bq_sb[:, 0, :], func=AF.Exp)

        def ps_tile(name):
            return psa.tile([128, 1024], F32, tag="sc", bufs=2, name=name)

        def emit_qkproj(X_sb, W_sb, b_sb, XPT, tci, p, tsl=None):
            # one 512-token chunk of a q/k projection for ONE pair
            if tsl is None:
                tsl = slice(tci * 512, tci * 512 + 512)
            n = tsl.stop - tsl.start
            ps_t = ps_tile(f"pj{p}")
            for kk in range(NK):
                nc.tensor.matmul(
                    ps_t[:, :n],
                    lhsT=W_sb[:, kk, p * 128 : p * 128 + 128],
                    rhs=X_sb[:, kk, tsl],
                    start=(kk == 0),
                    stop=(kk == NK - 1),
                )
            # bias-add gates the next scores' rhs: outrank the DVE z-chains
            with tc.high_priority():
                nc.vector.tensor_scalar_add(
                    XPT[:, p, tsl], ps_t[:, :n], b_sb[:, p, :]
                )

        def emit_vproj(tt):
            psv = ps_tile("projv")
            for kk in range(NK):
                nc.tensor.matmul(
                    psv[:, :DOUT],
                    lhsT=vT_sb[:, kk, tt * 128 : tt * 128 + 128],
                    rhs=wv_sb[:, kk, :],
                    start=(kk == 0),
                    stop=(kk == NK - 1),
                )
            nc.vector.scalar_tensor_tensor(
                out=vp_sb[:, tt, :],
                in0=psv[:, :DOUT],
                scalar=1.0,
                in1=bv_sb[:],
                op0=ALU.mult,
                op1=ALU.add,
            )

        def emit_scores(p, kb, qhs=None, scs=None):
            ksl = slice(kb * 128, kb * 128 + 128)
            if scs is None:
                scs = {}
            for qh in qhs if qhs is not None else range(NQH):
                for h in range(2):
                    sc = ps_tile(f"sc{h}{qh}")
                    for qq in range(2):
                        hsl = slice(h * 64, h * 64 + 64)
                        qsl = slice(
                            qh * 1024 + qq * 512, qh * 1024 + qq * 512 + 512
                        )
                        nc.tensor.matmul(
                            sc[:, qq * 512 : qq * 512 + 512],
                            lhsT=kpT_sb[hsl, p, ksl],
                            rhs=qpT_sb[hsl, p, qsl],
                            start=True,
                            stop=True,
                        )
                    scs[(h, qh)] = sc
            return scs

        def emit_exps(scs):
            # qh-major: the qh=0 exps only need the qh=0 score tiles, so
            # they clear the strict ACT FIFO before qh=1 scores are ready
            at_tiles, z_parts = {}, {}
            for qh in range(NQH):
                for h in range(2):
                    at = asb.tile(
                        [128, 1024], BF16, tag=f"at{h}_{qh}", bufs=4,
                        name=f"at{h}{qh}",
                    )
                    z = asb.tile(
                        [128, 1], F32, tag=f"z{h}_{qh}", bufs=4, name=f"z{h}{qh}"
                    )
                    nc.scalar.activation(
                        out=at[:], in_=scs[(h, qh)][:], func=AF.Exp,
                        accum_out=z[:],
                    )
                    at_tiles[(h, qh)] = at
                    z_parts[(h, qh)] = z
            return at_tiles, z_parts

        def emit_pv(p, kb, at_tiles, z_parts, pvts, hs=(0, 1)):
            # h=0 right after this kb's exps (its Z-chain gate is exp#3);
            # h=1 is software-pipelined one kb later: the PE is strictly
            # in-order, so a stalled pv-h1 would block everything behind it
            for h in hs:
                if NQH > 1:
                    zs = asb.tile([128, 1], F32, tag=f"zs{h}", bufs=2, name="zs")
                    nc.vector.tensor_add(
                        zs[:], z_parts[(h, 0)][:], z_parts[(h, 1)][:]
                    )
                    for qh in range(2, NQH):
                        nc.vector.tensor_add(zs[:], zs[:], z_parts[(h, qh)][:])
                else:
                    zs = z_parts[(h, 0)]
                rz = asb.tile([128, 1], F32, tag=f"rz{h}", bufs=2, name="rz")
                nc.vector.reciprocal(rz[:], zs[:])
                vhs = asb.tile([128, HD], BF16, tag=f"vh{h}", bufs=2, name="vhs")
                nc.vector.tensor_scalar_mul(
                    vhs[:],
                    vp_sb[:, kb, p * 128 + h * 64 : p * 128 + h * 64 + 64],
                    rz[:],
                )
                # accumulate in PSUM across all kb; one accumulation group
                # per bank (start only at kb==0 h==0, stop at the very end)
                for qh in range(NQH):
                    for qq in range(2):
                        nc.tensor.matmul(
                            pvts[qh][
                                h * 64 : h * 64 + 64,
                                qq * 512 : qq * 512 + 512,
                            ],
                            lhsT=vhs[:],
                            rhs=at_tiles[(h, qh)][
                                :, qq * 512 : qq * 512 + 512
                            ],
                            start=(kb == 0),
                            stop=(kb == NKB - 1),
                            tile_position=(0, h * 64),
                            skip_group_check=True,
                        )

        def emit_oproj(p, tt, tail=False):
            # one token tile of pair-p's O-projection partial
            ost = osb.tile([128, D], F16, tag="ost", bufs=6, name="ost")
            if tail:
                # pvt banks are drained by now: rotate through 3 PSUM tags
                # for a deeper pipeline
                tag = ("sc", "pvt0", "pvt1")[tt % 3]
                if tag == "sc":
                    ps_t = ps_tile(f"o{tt}")
                else:
                    ps_t = psa.tile([128, 1024], F32, tag=tag, bufs=1, name=f"o{tt}")
            else:
                ps_t = ps_tile(f"o{tt % 2}")
            for dc in range(2):
                nc.tensor.matmul(
                    ps_t[:, dc * 512 : dc * 512 + 512],
                    lhsT=hcT_sb[:, p, tt * 128 : tt * 128 + 128],
                    rhs=wo_sb[:, p, dc * 512 : dc * 512 + 512],
                    start=True,
                    stop=True,
                )
            if tail:
                # ACT is idle at the tail: split the copy across both engines
                nc.vector.tensor_copy(ost[:, 0:512], ps_t[:, 0:512])
                nc.scalar.copy(ost[:, 512:1024], ps_t[:, 512:1024])
            else:
                nc.vector.tensor_copy(ost[:], ps_t[:])
            nc.sync.dma_start(out=outv[p][tt], in_=ost[:])

        # ---- remaining DMA waves, ordered by consumption time ----
        for kk in range(NK):  # v tokens 0:512 -> v-proj tiles 0..3
            nc.sync.dma_start(out=vT_sb[:, kk, s0], in_=vTv[kk][:, s0])
        for sl in (slice(512, 1024), slice(1024, 1536), slice(1536, 2048)):
            for kk in range(NK):
                nc.sync.dma_start(out=kT_sb[:, kk, sl], in_=kTv[kk][:, sl])
            for kk in range(NK):
                nc.sync.dma_start(out=vT_sb[:, kk, sl], in_=vTv[kk][:, sl])
        nc.sync.dma_start(out=wo_sb[:], in_=wov)

        # ---- head, ordered to match DMA arrival: q chunks 0,1 then the
        # first k chunk, then the qh=0 scores of kb0 (so the first exps can
        # fire), then q chunks 2,3 and the qh=1 scores ----
        emit_qkproj(qT_sb, wq_sb, bq_sb, qpT_sb, 0, 0)
        emit_qkproj(qT_sb, wq_sb, bq_sb, qpT_sb, 1, 0)
        emit_qkproj(kT_sb, wk_sb, bk_sb, kpT_sb, 0, 0)
        scs0 = {}
        emit_scores(0, 0, qhs=[0], scs=scs0)
        emit_qkproj(qT_sb, wq_sb, bq_sb, qpT_sb, 2, 0)
        emit_qkproj(qT_sb, wq_sb, bq_sb, qpT_sb, 3, 0)
        emit_scores(0, 0, qhs=[1], scs=scs0)

        # remaining projection chunks: kb -> chunk.  qk chunks sit ALONE on
        # their kb (sc-rotation starves with vproj+qkproj on one kb); vproj
        # runs in pairs on the other kbs, always ahead of its PV deadline.
        proj_sched = {
            2: (kT_sb, wk_sb, bk_sb, kpT_sb, 1, 0),
            4: (kT_sb, wk_sb, bk_sb, kpT_sb, 2, 0),
            6: (kT_sb, wk_sb, bk_sb, kpT_sb, 3, 0),
            8: (qT_sb, wq_sb, bq_sb, qpT_sb, 0, 1),
            9: (qT_sb, wq_sb, bq_sb, qpT_sb, 1, 1),
            10: (qT_sb, wq_sb, bq_sb, qpT_sb, 2, 1),
            11: (qT_sb, wq_sb, bq_sb, qpT_sb, 3, 1),
            12: (kT_sb, wk_sb, bk_sb, kpT_sb, 0, 1),
        }
        # k-p1 chunks 1..3 are not needed until p1 kbs 4/8/12: emit them
        # inside the p1 phase to rebalance PE load between the two phases
        p1_sched = {
            1: (kT_sb, wk_sb, bk_sb, kpT_sb, 1, 1),
            5: (kT_sb, wk_sb, bk_sb, kpT_sb, 2, 1),
            9: (kT_sb, wk_sb, bk_sb, kpT_sb, 3, 1),
        }

        # resident PV accumulators (allocated once, reused across pairs)
        pvts = {
            qh: psa.tile([128, 1024], F32, tag=f"pvt{qh}", bufs=1, name=f"pvt{qh}")
            for qh in range(NQH)
        }

        def attention(p, per_kb_extra, scs=None):
            nonlocal pvts
            if scs is None:
                with tc.high_priority():
                    scs = emit_scores(p, 0)
            if p == 0:
                # after the first scores: emitting it earlier would push the
                # first exp's PE-counter wait past the vT DMA arrival
                emit_vproj(0)
            pending = None
            for kb in range(NKB):
                at_tiles, z_parts = emit_exps(scs)
                if pending is not None:
                    # previous kb's h=1 PV: its gate (exp4 + z-chain of the
                    # previous window) is long past, so it runs stall-free
                    emit_pv(p, kb - 1, *pending, pvts, hs=(1,))
                per_kb_extra(kb)
                if kb + 1 < NKB:
                    # allocation order stays extras-first (good WAR
                    # aliasing) but the scores matmuls get top scheduler
                    # priority: they feed the ACT exp stream, which ends
                    # the kernel — extras can always wait
                    with tc.high_priority():
                        scs = emit_scores(p, kb + 1)
                emit_pv(p, kb, at_tiles, z_parts, pvts, hs=(0,))
                pending = (at_tiles, z_parts)
            emit_pv(p, NKB - 1, *pending, pvts, hs=(1,))
            # drain PV accumulators -> fp16 O-proj lhsT; high priority:
            # these copies gate every O-projection tile of this pair
            with tc.high_priority():
                for qh in range(NQH):
                    pvt = pvts[qh]
                    qsl = slice(qh * 1024, qh * 1024 + 1024)
                    nc.vector.tensor_copy(hcT_sb[:, p, qsl], pvt[:])
            if p == 0:
                # re-allocate the same tags for pair 1 (WAR via pool deps)
                pvts = {
                    qh: psa.tile(
                        [128, 1024], F32, tag=f"pvt{qh}", bufs=1, name=f"pvt{qh}b"
                    )
                    for qh in range(NQH)
                }

        def p0_extra(kb):
            if kb + 1 < NKB:
                emit_vproj(kb + 1)
            if kb in proj_sched:
                emit_qkproj(*proj_sched.pop(kb))

        attention(0, p0_extra, scs=scs0)
        for kb in sorted(proj_sched):
            emit_qkproj(*proj_sched.pop(kb))

        def p1_extra(kb):
            # O-proj pairs on even kbs (empirically best of the tested
            # phase/parity layouts)
            if kb % 2 == 0:
                emit_oproj(0, kb)
                emit_oproj(0, kb + 1)
            if kb in p1_sched:
                emit_qkproj(*p1_sched.pop(kb))

        attention(1, p1_extra)

        # ---- tail: pair-1 O-projection ----
        for tt in range(NTT):
            emit_oproj(1, tt, tail=True)

    nc.compile()
    return nc


# ---------------- host-side shard / unshard ----------------

S = 2048
B = 2

_NC_CACHE = {}


def _get_nc():
    if "nc" not in _NC_CACHE:
        _NC_CACHE["nc"] = build_kernel(S=S)
    return _NC_CACHE["nc"]


def make_in_maps(q, k, v, Wq, bq, Wk, bk, Wv, bv, Wo, bo):
    bf = ml_dtypes.bfloat16
    f16 = np.float16
    maps = []
    for c in range(8):
        b = c // 4
        hc = c % 4
        cols = slice(256 * hc, 256 * hc + 256)
        maps.append({
            "qT": np.ascontiguousarray(q[b].T.astype(f16)),
            "kT": np.ascontiguousarray(k[b].T.astype(f16)),
            "vT": np.ascontiguousarray(v[b].astype(bf).T),
            "wq": np.ascontiguousarray(Wq[:, cols].astype(f16)),
            "wk": np.ascontiguousarray(Wk[:, cols].astype(f16)),
            "wv": np.ascontiguousarray(Wv[:, cols].astype(bf)),
            "wo": np.ascontiguousarray(Wo[cols, :].astype(f16)),
            "bq": np.ascontiguousarray(
                bq[cols].reshape(NPAIR, 128, 1).astype(np.float32)
            ),
            "bk": np.ascontiguousarray(
                bk[cols].reshape(NPAIR, 128, 1).astype(np.float32)
            ),
            "bv": np.ascontiguousarray(bv[cols].astype(np.float32)),
        })
    return maps


def kernel(q, k, v, Wq, bq, Wk, bk, Wv, bv, Wo, bo):
    q = np.asarray(q, dtype=np.float32)
    k = np.asarray(k, dtype=np.float32)
    v = np.asarray(v, dtype=np.float32)
    Wq = np.asarray(Wq, dtype=np.float32)
    Wk = np.asarray(Wk, dtype=np.float32)
    Wv = np.asarray(Wv, dtype=np.float32)
    Wo = np.asarray(Wo, dtype=np.float32)
    bq = np.asarray(bq, dtype=np.float32)
    bk = np.asarray(bk, dtype=np.float32)
    bv = np.asarray(bv, dtype=np.float32)
    bo = np.asarray(bo, dtype=np.float32)

    nc = _get_nc()
    maps = make_in_maps(q, k, v, Wq, bq, Wk, bk, Wv, bv, Wo, bo)
    res = run_bass_kernel_spmd(nc, maps, core_ids=list(range(8)))

    outs = []
    for b in range(B):
        acc = np.zeros((S, D), dtype=np.float32)
        for hc in range(4):
            r = res.results[b * 4 + hc]
            acc += r["out0"].astype(np.float32)
            acc += r["out1"].astype(np.float32)
        acc += bo[None, :]
        outs.append(acc)
    return np.stack(outs, axis=0)


# revision 81
# speedup vs baseline: 1.0003x; 1.0003x over previous
"""Bass/Tile kernel for nn_MultiHeadAttention (B=2, S=2048, D=1024, H=16).

Sharding: 8 cores = 2 (batch) x 4 (head-chunks of 4 heads).
Each core computes, for its batch b and its 4 heads (2 pairs of 2):
  qpT/kpT = (x @ W{q,k} + b)^T  in [dout, token] fp16 layout
  vp      = v @ Wv + bv         in [token, dout] bf16 layout
  scoresT = kp @ qp^T           per head, [k, q] fp32 PSUM
  attnT   = exp(scoresT)        (softmax over q == free axis) -> bf16
  Z[k]    = sum_q attnT[k, q]   (ACT accum_out, fp32)
  outT    = sum_kb (vp[kb]/Z[kb]) PV matmuls, accumulated IN PSUM
            across all 16 k-blocks (pvt0/pvt1 resident banks)
  out_p   = hcT_p^T @ Wo_p      per-pair fp16 partials (host sums 8)

PSUM: sc tag bufs=2 (4 banks, shared by scores AND all projection
chunks) + pvt0/pvt1 resident accumulators (4 banks) = 8.
Schedule: pair-0 q/k proj head; v-proj + remaining projections trickle
through pair-0 attention; pair-0's O-projection trickles through
pair-1 attention; tail is only pair-1's O-projection.
"""

import sys

sys.path.insert(0, "/opt/trn_rl_repo")

from contextlib import ExitStack

import numpy as np
import ml_dtypes

import concourse.bass as bass
import concourse.mybir as mybir
import concourse.tile as tile
from concourse import bacc
from concourse.bass_utils import run_bass_kernel_spmd

BF16 = mybir.dt.bfloat16
F16 = mybir.dt.float16
F32 = mybir.dt.float32
AF = mybir.ActivationFunctionType
ALU = mybir.AluOpType

D = 1024
NK = 8  # k-tiles over D
DOUT = 256  # per-core head dims (4 heads)
NPAIR = 2  # pairs of heads (128 dout each)
HD = 64


def build_kernel(S=2048):
    NKB = S // 128  # k-token blocks
    NQH = S // 1024  # exp blocks of 1024 along q
    NTC = S // 512  # proj token chunks
    NTT = S // 128  # token tiles
    assert S % 1024 == 0

    nc = bacc.Bacc("TRN2", target_bir_lowering=False, debug=False)

    qT = nc.dram_tensor("qT", [D, S], F16, kind="ExternalInput")
    kT = nc.dram_tensor("kT", [D, S], F16, kind="ExternalInput")
    vT = nc.dram_tensor("vT", [D, S], BF16, kind="ExternalInput")
    wq = nc.dram_tensor("wq", [D, DOUT], F16, kind="ExternalInput")
    wk = nc.dram_tensor("wk", [D, DOUT], F16, kind="ExternalInput")
    wv = nc.dram_tensor("wv", [D, DOUT], BF16, kind="ExternalInput")
    wo = nc.dram_tensor("wo", [DOUT, D], F16, kind="ExternalInput")
    bq = nc.dram_tensor("bq", [NPAIR, 128, 1], F32, kind="ExternalInput")
    bk = nc.dram_tensor("bk", [NPAIR, 128, 1], F32, kind="ExternalInput")
    bv = nc.dram_tensor("bv", [DOUT], F32, kind="ExternalInput")
    out0 = nc.dram_tensor("out0", [S, D], F16, kind="ExternalOutput")
    out1 = nc.dram_tensor("out1", [S, D], F16, kind="ExternalOutput")

    # tiled DRAM views
    qTv = qT.ap().rearrange("(t p) s -> t p s", p=128)  # [8, 128, S]
    kTv = kT.ap().rearrange("(t p) s -> t p s", p=128)
    vTv = vT.ap().rearrange("(t p) s -> t p s", p=128)
    wqv = wq.ap().rearrange("(t p) m -> p t m", p=128)  # [128, 8, 256]
    wkv = wk.ap().rearrange("(t p) m -> p t m", p=128)
    wvv = wv.ap().rearrange("(t p) m -> p t m", p=128)
    wov = wo.ap().rearrange("(t p) m -> p t m", p=128)  # [128, 2, 1024]
    bqv = bq.ap().rearrange("a p o -> p a o")  # [128, 2, 1]
    bkv = bk.ap().rearrange("a p o -> p a o")
    outv = [
        out0.ap().rearrange("(t p) m -> t p m", p=128),  # [NTT, 128, 1024]
        out1.ap().rearrange("(t p) m -> t p m", p=128),
    ]

    bv_bcast_ap = bass.AP(tensor=bv.ap().tensor, offset=0, ap=[[0, 128], [1, DOUT]])

    with tile.TileContext(nc) as tc, ExitStack() as ctx:
        sb = ctx.enter_context(tc.tile_pool(name="sb", bufs=1))

        # ---- resident weight/bias loads ----
        wq_sb = sb.tile([128, NK, DOUT], F16, tag="wq")
        wk_sb = sb.tile([128, NK, DOUT], F16, tag="wk")
        wv_sb = sb.tile([128, NK, DOUT], BF16, tag="wv")
        wo_sb = sb.tile([128, NPAIR, D], F16, tag="wo")
        bq_sb = sb.tile([128, NPAIR, 1], F32, tag="bq")
        bk_sb = sb.tile([128, NPAIR, 1], F32, tag="bk")
        bv_sb = sb.tile([128, DOUT], F32, tag="bv")

        # ---- resident activations ----
        qT_sb = sb.tile([128, NK, S], F16, tag="qT")
        kT_sb = sb.tile([128, NK, S], F16, tag="kT")
        vT_sb = sb.tile([128, NK, S], BF16, tag="vT")

        # head loads, ordered by first use; big tensors split into
        # token-slices so compute starts on partial data (DMA engines
        # serialize in practice)
        nc.sync.dma_start(out=bq_sb[:], in_=bqv)
        nc.sync.dma_start(out=wq_sb[:], in_=wqv)
        s0 = slice(0, 512)
        for kk in range(NK):  # q tokens 0:1024 -> proj chunks 0,1
            nc.sync.dma_start(out=qT_sb[:, kk, 0:1024], in_=qTv[kk][:, 0:1024])
        # k-path weights aren't needed until the k-proj chunk (~11us in)
        nc.sync.dma_start(out=wk_sb[:], in_=wkv)
        nc.sync.dma_start(out=bk_sb[:], in_=bkv)
        for kk in range(NK):  # k tokens 0:512 -> k-proj chunk 0
            nc.sync.dma_start(out=kT_sb[:, kk, s0], in_=kTv[kk][:, s0])
        for kk in range(NK):  # q tokens 1024:2048 -> proj chunks 2,3
            nc.sync.dma_start(out=qT_sb[:, kk, 1024:2048], in_=qTv[kk][:, 1024:2048])
        # v-path weights are not needed until v-proj (~20us in)
        nc.sync.dma_start(out=wv_sb[:], in_=wvv)
        nc.sync.dma_start(out=bv_sb[:], in_=bv_bcast_ap)

        # ---- projection outputs (resident SBUF) ----
        qpT_sb = sb.tile([128, NPAIR, S], F16, tag="qpT")
        kpT_sb = sb.tile([128, NPAIR, S], F16, tag="kpT")
        vp_sb = sb.tile([128, NTT, DOUT], BF16, tag="vp")
        hcT_sb = sb.tile([128, NPAIR, S], F16, tag="hcT")

        # PSUM: sc bufs=2 (4 banks, scores + all projections) +
        # pvt0/pvt1 resident PV accumulators (4 banks) = 8
        psa = ctx.enter_context(tc.tile_pool(name="ps_all", bufs=1, space="PSUM"))
        asb = ctx.enter_context(tc.tile_pool(name="att_sb", bufs=1))
        osb = ctx.enter_context(tc.tile_pool(name="o_sb", bufs=1))

        # warm the exp table during head DMA (table load ~2.7us)
        warm = sb.tile([128, 1], F32, tag="warm")
        nc.scalar.activation(out=warm[:], in_=bq_sb[:, 0, :], func=AF.Exp)

        def ps_tile(name):
            return psa.tile([128, 1024], F32, tag="sc", bufs=2, name=name)

        def emit_qkproj(X_sb, W_sb, b_sb, XPT, tci, p, tsl=None):
            # one 512-token chunk of a q/k projection for ONE pair
            if tsl is None:
                tsl = slice(tci * 512, tci * 512 + 512)
            n = tsl.stop - tsl.start
            ps_t = ps_tile(f"pj{p}")
            for kk in range(NK):
                nc.tensor.matmul(
                    ps_t[:, :n],
                    lhsT=W_sb[:, kk, p * 128 : p * 128 + 128],
                    rhs=X_sb[:, kk, tsl],
                    start=(kk == 0),
                    stop=(kk == NK - 1),
                )
            # bias-add gates the next scores' rhs: outrank the DVE z-chains
            with tc.high_priority():
                nc.vector.tensor_scalar_add(
                    XPT[:, p, tsl], ps_t[:, :n], b_sb[:, p, :]
                )

        def emit_vproj(tt):
            psv = ps_tile("projv")
            for kk in range(NK):
                nc.tensor.matmul(
                    psv[:, :DOUT],
                    lhsT=vT_sb[:, kk, tt * 128 : tt * 128 + 128],
                    rhs=wv_sb[:, kk, :],
                    start=(kk == 0),
                    stop=(kk == NK - 1),
                )
            nc.vector.scalar_tensor_tensor(
                out=vp_sb[:, tt, :],
                in0=psv[:, :DOUT],
                scalar=1.0,
                in1=bv_sb[:],
                op0=ALU.mult,
                op1=ALU.add,
            )

        def emit_scores(p, kb, qhs=None, scs=None):
            ksl = slice(kb * 128, kb * 128 + 128)
            if scs is None:
                scs = {}
            for qh in qhs if qhs is not None else range(NQH):
                for h in range(2):
                    sc = ps_tile(f"sc{h}{qh}")
                    for qq in range(2):
                        hsl = slice(h * 64, h * 64 + 64)
                        qsl = slice(
                            qh * 1024 + qq * 512, qh * 1024 + qq * 512 + 512
                        )
                        nc.tensor.matmul(
                            sc[:, qq * 512 : qq * 512 + 512],
                            lhsT=kpT_sb[hsl, p, ksl],
                            rhs=qpT_sb[hsl, p, qsl],
                            start=True,
                            stop=True,
                        )
                    scs[(h, qh)] = sc
            return scs

        def emit_exps(scs):
            # qh-major: the qh=0 exps only need the qh=0 score tiles, so
            # they clear the strict ACT FIFO before qh=1 scores are ready
            at_tiles, z_parts = {}, {}
            for qh in range(NQH):
                for h in range(2):
                    at = asb.tile(
                        [128, 1024], BF16, tag=f"at{h}_{qh}", bufs=5,
                        name=f"at{h}{qh}",
                    )
                    z = asb.tile(
                        [128, 1], F32, tag=f"z{h}_{qh}", bufs=5, name=f"z{h}{qh}"
                    )
                    nc.scalar.activation(
                        out=at[:], in_=scs[(h, qh)][:], func=AF.Exp,
                        accum_out=z[:],
                    )
                    at_tiles[(h, qh)] = at
                    z_parts[(h, qh)] = z
            return at_tiles, z_parts

        def emit_pv(p, kb, at_tiles, z_parts, pvts, hs=(0, 1)):
            # h=0 right after this kb's exps (its Z-chain gate is exp#3);
            # h=1 is software-pipelined one kb later: the PE is strictly
            # in-order, so a stalled pv-h1 would block everything behind it
            for h in hs:
                if NQH > 1:
                    zs = asb.tile([128, 1], F32, tag=f"zs{h}", bufs=2, name="zs")
                    nc.vector.tensor_add(
                        zs[:], z_parts[(h, 0)][:], z_parts[(h, 1)][:]
                    )
                    for qh in range(2, NQH):
                        nc.vector.tensor_add(zs[:], zs[:], z_parts[(h, qh)][:])
                else:
                    zs = z_parts[(h, 0)]
                rz = asb.tile([128, 1], F32, tag=f"rz{h}", bufs=2, name="rz")
                nc.vector.reciprocal(rz[:], zs[:])
                vhs = asb.tile([128, HD], BF16, tag=f"vh{h}", bufs=2, name="vhs")
                nc.vector.tensor_scalar_mul(
                    vhs[:],
                    vp_sb[:, kb, p * 128 + h * 64 : p * 128 + h * 64 + 64],
                    rz[:],
                )
                # accumulate in PSUM across all kb; one accumulation group
                # per bank (start only at kb==0 h==0, stop at the very end)
                for qh in range(NQH):
                    for qq in range(2):
                        nc.tensor.matmul(
                            pvts[qh][
                                h * 64 : h * 64 + 64,
                                qq * 512 : qq * 512 + 512,
                            ],
                            lhsT=vhs[:],
                            rhs=at_tiles[(h, qh)][
                                :, qq * 512 : qq * 512 + 512
                            ],
                            start=(kb == 0),
                            stop=(kb == NKB - 1),
                            tile_position=(0, h * 64),
                            skip_group_check=True,
                        )

        def emit_oproj(p, tt, tail=False):
            # one token tile of pair-p's O-projection partial
            ost = osb.tile([128, D], F16, tag="ost", bufs=6, name="ost")
            if tail:
                # pvt banks are drained by now: rotate through 3 PSUM tags
                # for a deeper pipeline
                tag = ("sc", "pvt0", "pvt1")[tt % 3]
                if tag == "sc":
                    ps_t = ps_tile(f"o{tt}")
                else:
                    ps_t = psa.tile([128, 1024], F32, tag=tag, bufs=1, name=f"o{tt}")
            else:
                ps_t = ps_tile(f"o{tt % 2}")
            for dc in range(2):
                nc.tensor.matmul(
                    ps_t[:, dc * 512 : dc * 512 + 512],
                    lhsT=hcT_sb[:, p, tt * 128 : tt * 128 + 128],
                    rhs=wo_sb[:, p, dc * 512 : dc * 512 + 512],
                    start=True,
                    stop=True,
                )
            if tail:
                # ACT is idle at the tail: split the copy across both engines
                nc.vector.tensor_copy(ost[:, 0:512], ps_t[:, 0:512])
                nc.scalar.copy(ost[:, 512:1024], ps_t[:, 512:1024])
            else:
                nc.vector.tensor_copy(ost[:], ps_t[:])
            nc.sync.dma_start(out=outv[p][tt], in_=ost[:])

        # ---- remaining DMA waves, ordered by consumption time ----
        for kk in range(NK):  # v tokens 0:512 -> v-proj tiles 0..3
            nc.sync.dma_start(out=vT_sb[:, kk, s0], in_=vTv[kk][:, s0])
        for sl in (slice(512, 1024), slice(1024, 1536), slice(1536, 2048)):
            for kk in range(NK):
                nc.sync.dma_start(out=kT_sb[:, kk, sl], in_=kTv[kk][:, sl])
            for kk in range(NK):
                nc.sync.dma_start(out=vT_sb[:, kk, sl], in_=vTv[kk][:, sl])
        nc.sync.dma_start(out=wo_sb[:], in_=wov)

        # ---- head, ordered to match DMA arrival: q chunks 0,1 then the
        # first k chunk, then the qh=0 scores of kb0 (so the first exps can
        # fire), then q chunks 2,3 and the qh=1 scores ----
        emit_qkproj(qT_sb, wq_sb, bq_sb, qpT_sb, 0, 0)
        emit_qkproj(qT_sb, wq_sb, bq_sb, qpT_sb, 1, 0)
        emit_qkproj(kT_sb, wk_sb, bk_sb, kpT_sb, 0, 0)
        scs0 = {}
        emit_scores(0, 0, qhs=[0], scs=scs0)
        emit_qkproj(qT_sb, wq_sb, bq_sb, qpT_sb, 2, 0)
        emit_qkproj(qT_sb, wq_sb, bq_sb, qpT_sb, 3, 0)
        emit_scores(0, 0, qhs=[1], scs=scs0)

        # remaining projection chunks: kb -> chunk.  qk chunks sit ALONE on
        # their kb (sc-rotation starves with vproj+qkproj on one kb); vproj
        # runs in pairs on the other kbs, always ahead of its PV deadline.
        proj_sched = {
            2: (kT_sb, wk_sb, bk_sb, kpT_sb, 1, 0),
            4: (kT_sb, wk_sb, bk_sb, kpT_sb, 2, 0),
            6: (kT_sb, wk_sb, bk_sb, kpT_sb, 3, 0),
            8: (qT_sb, wq_sb, bq_sb, qpT_sb, 0, 1),
            9: (qT_sb, wq_sb, bq_sb, qpT_sb, 1, 1),
            10: (qT_sb, wq_sb, bq_sb, qpT_sb, 2, 1),
            11: (qT_sb, wq_sb, bq_sb, qpT_sb, 3, 1),
            12: (kT_sb, wk_sb, bk_sb, kpT_sb, 0, 1),
        }
        # k-p1 chunks 1..3 are not needed until p1 kbs 4/8/12: emit them
        # inside the p1 phase to rebalance PE load between the two phases
        p1_sched = {
            1: (kT_sb, wk_sb, bk_sb, kpT_sb, 1, 1),
            5: (kT_sb, wk_sb, bk_sb, kpT_sb, 2, 1),
            9: (kT_sb, wk_sb, bk_sb, kpT_sb, 3, 1),
        }

        # resident PV accumulators (allocated once, reused across pairs)
        pvts = {
            qh: psa.tile([128, 1024], F32, tag=f"pvt{qh}", bufs=1, name=f"pvt{qh}")
            for qh in range(NQH)
        }

        def attention(p, per_kb_extra, scs=None):
            nonlocal pvts
            if scs is None:
                with tc.high_priority():
                    scs = emit_scores(p, 0)
            if p == 0:
                # after the first scores: emitting it earlier would push the
                # first exp's PE-counter wait past the vT DMA arrival
                emit_vproj(0)
            pending = None
            for kb in range(NKB):
                at_tiles, z_parts = emit_exps(scs)
                if pending is not None:
                    # previous kb's h=1 PV: its gate (exp4 + z-chain of the
                    # previous window) is long past, so it runs stall-free
                    emit_pv(p, kb - 1, *pending, pvts, hs=(1,))
                per_kb_extra(kb)
                if kb + 1 < NKB:
                    # allocation order stays extras-first (good WAR
                    # aliasing) but the scores matmuls get top scheduler
                    # priority: they feed the ACT exp stream, which ends
                    # the kernel — extras can always wait
                    with tc.high_priority():
                        scs = emit_scores(p, kb + 1)
                emit_pv(p, kb, at_tiles, z_parts, pvts, hs=(0,))
                pending = (at_tiles, z_parts)
            emit_pv(p, NKB - 1, *pending, pvts, hs=(1,))
            # drain PV accumulators -> fp16 O-proj lhsT; high priority:
            # these copies gate every O-projection tile of this pair
            with tc.high_priority():
                for qh in range(NQH):
                    pvt = pvts[qh]
                    qsl = slice(qh * 1024, qh * 1024 + 1024)
                    nc.vector.tensor_copy(hcT_sb[:, p, qsl], pvt[:])
            if p == 0:
                # re-allocate the same tags for pair 1 (WAR via pool deps)
                pvts = {
                    qh: psa.tile(
                        [128, 1024], F32, tag=f"pvt{qh}", bufs=1, name=f"pvt{qh}b"
                    )
                    for qh in range(NQH)
                }

        def p0_extra(kb):
            if kb + 1 < NKB:
                emit_vproj(kb + 1)
            if kb in proj_sched:
                emit_qkproj(*proj_sched.pop(kb))

        attention(0, p0_extra, scs=scs0)
        for kb in sorted(proj_sched):
            emit_qkproj(*proj_sched.pop(kb))

        def p1_extra(kb):
            # O-proj pairs on even kbs (empirically best of the tested
            # phase/parity layouts)
            if kb % 2 == 0:
                emit_oproj(0, kb)
                emit_oproj(0, kb + 1)
            if kb in p1_sched:
                emit_qkproj(*p1_sched.pop(kb))

        attention(1, p1_extra)

        # ---- tail: pair-1 O-projection ----
        for tt in range(NTT):
            emit_oproj(1, tt, tail=True)

    nc.compile()
    return nc


# ---------------- host-side shard / unshard ----------------

S = 2048
B = 2

_NC_CACHE = {}


def _get_nc():
    if "nc" not in _NC_CACHE:
        _NC_CACHE["nc"] = build_kernel(S=S)
    return _NC_CACHE["nc"]


def make_in_maps(q, k, v, Wq, bq, Wk, bk, Wv, bv, Wo, bo):
    bf = ml_dtypes.bfloat16
    f16 = np.float16
    maps = []
    for c in range(8):
        b = c // 4
        hc = c % 4
        cols = slice(256 * hc, 256 * hc + 256)
        maps.append({
            "qT": np.ascontiguousarray(q[b].T.astype(f16)),
            "kT": np.ascontiguousarray(k[b].T.astype(f16)),
            "vT": np.ascontiguousarray(v[b].astype(bf).T),
            "wq": np.ascontiguousarray(Wq[:, cols].astype(f16)),
            "wk": np.ascontiguousarray(Wk[:, cols].astype(f16)),
            "wv": np.ascontiguousarray(Wv[:, cols].astype(bf)),
            "wo": np.ascontiguousarray(Wo[cols, :].astype(f16)),
            "bq": np.ascontiguousarray(
                bq[cols].reshape(NPAIR, 128, 1).astype(np.float32)
            ),
            "bk": np.ascontiguousarray(
                bk[cols].reshape(NPAIR, 128, 1).astype(np.float32)
            ),
            "bv": np.ascontiguousarray(bv[cols].astype(np.float32)),
        })
    return maps


def kernel(q, k, v, Wq, bq, Wk, bk, Wv, bv, Wo, bo):
    q = np.asarray(q, dtype=np.float32)
    k = np.asarray(k, dtype=np.float32)
    v = np.asarray(v, dtype=np.float32)
    Wq = np.asarray(Wq, dtype=np.float32)
    Wk = np.asarray(Wk, dtype=np.float32)
    Wv = np.asarray(Wv, dtype=np.float32)
    Wo = np.asarray(Wo, dtype=np.float32)
    bq = np.asarray(bq, dtype=np.float32)
    bk = np.asarray(bk, dtype=np.float32)
    bv = np.asarray(bv, dtype=np.float32)
    bo = np.asarray(bo, dtype=np.float32)

    nc = _get_nc()
    maps = make_in_maps(q, k, v, Wq, bq, Wk, bk, Wv, bv, Wo, bo)
    res = run_bass_kernel_spmd(nc, maps, core_ids=list(range(8)))

    outs = []
    for b in range(B):
        acc = np.zeros((S, D), dtype=np.float32)
        for hc in range(4):
            r = res.results[b * 4 + hc]
            acc += r["out0"].astype(np.float32)
            acc += r["out1"].astype(np.float32)
        acc += bo[None, :]
        outs.append(acc)
    return np.stack(outs, axis=0)


# revision 86
# speedup vs baseline: 1.0017x; 1.0014x over previous
"""Bass/Tile kernel for nn_MultiHeadAttention (B=2, S=2048, D=1024, H=16).

Sharding: 8 cores = 2 (batch) x 4 (head-chunks of 4 heads).
Each core computes, for its batch b and its 4 heads (2 pairs of 2):
  qpT/kpT = (x @ W{q,k} + b)^T  in [dout, token] fp16 layout
  vp      = v @ Wv + bv         in [token, dout] bf16 layout
  scoresT = kp @ qp^T           per head, [k, q] fp32 PSUM
  attnT   = exp(scoresT)        (softmax over q == free axis) -> bf16
  Z[k]    = sum_q attnT[k, q]   (ACT accum_out, fp32)
  outT    = sum_kb (vp[kb]/Z[kb]) PV matmuls, accumulated IN PSUM
            across all 16 k-blocks (pvt0/pvt1 resident banks)
  out_p   = hcT_p^T @ Wo_p      per-pair fp16 partials (host sums 8)

PSUM: sc tag bufs=2 (4 banks, shared by scores AND all projection
chunks) + pvt0/pvt1 resident accumulators (4 banks) = 8.
Schedule: pair-0 q/k proj head; v-proj + remaining projections trickle
through pair-0 attention; pair-0's O-projection trickles through
pair-1 attention; tail is only pair-1's O-projection.
"""

import sys

sys.path.insert(0, "/opt/trn_rl_repo")

from contextlib import ExitStack

import numpy as np
import ml_dtypes

import concourse.bass as bass
import concourse.mybir as mybir
import concourse.tile as tile
from concourse import bacc
from concourse.bass_utils import run_bass_kernel_spmd

BF16 = mybir.dt.bfloat16
F16 = mybir.dt.float16
F32 = mybir.dt.float32
AF = mybir.ActivationFunctionType
ALU = mybir.AluOpType

D = 1024
NK = 8  # k-tiles over D
DOUT = 256  # per-core head dims (4 heads)
NPAIR = 2  # pairs of heads (128 dout each)
HD = 64


def build_kernel(S=2048):
    NKB = S // 128  # k-token blocks
    NQH = S // 1024  # exp blocks of 1024 along q
    NTC = S // 512  # proj token chunks
    NTT = S // 128  # token tiles
    assert S % 1024 == 0

    nc = bacc.Bacc("TRN2", target_bir_lowering=False, debug=False)

    qT = nc.dram_tensor("qT", [D, S], F16, kind="ExternalInput")
    kT = nc.dram_tensor("kT", [D, S], F16, kind="ExternalInput")
    vT = nc.dram_tensor("vT", [D, S], BF16, kind="ExternalInput")
    wq = nc.dram_tensor("wq", [D, DOUT], F16, kind="ExternalInput")
    wk = nc.dram_tensor("wk", [D, DOUT], F16, kind="ExternalInput")
    wv = nc.dram_tensor("wv", [D, DOUT], BF16, kind="ExternalInput")
    wo = nc.dram_tensor("wo", [DOUT, D], F16, kind="ExternalInput")
    bq = nc.dram_tensor("bq", [NPAIR, 128, 1], F32, kind="ExternalInput")
    bk = nc.dram_tensor("bk", [NPAIR, 128, 1], F32, kind="ExternalInput")
    bv = nc.dram_tensor("bv", [DOUT], F32, kind="ExternalInput")
    out0 = nc.dram_tensor("out0", [S, D], F16, kind="ExternalOutput")
    out1 = nc.dram_tensor("out1", [S, D], F16, kind="ExternalOutput")

    # tiled DRAM views
    qTv = qT.ap().rearrange("(t p) s -> t p s", p=128)  # [8, 128, S]
    kTv = kT.ap().rearrange("(t p) s -> t p s", p=128)
    vTv = vT.ap().rearrange("(t p) s -> t p s", p=128)
    wqv = wq.ap().rearrange("(t p) m -> p t m", p=128)  # [128, 8, 256]
    wkv = wk.ap().rearrange("(t p) m -> p t m", p=128)
    wvv = wv.ap().rearrange("(t p) m -> p t m", p=128)
    wov = wo.ap().rearrange("(t p) m -> p t m", p=128)  # [128, 2, 1024]
    bqv = bq.ap().rearrange("a p o -> p a o")  # [128, 2, 1]
    bkv = bk.ap().rearrange("a p o -> p a o")
    outv = [
        out0.ap().rearrange("(t p) m -> t p m", p=128),  # [NTT, 128, 1024]
        out1.ap().rearrange("(t p) m -> t p m", p=128),
    ]

    bv_bcast_ap = bass.AP(tensor=bv.ap().tensor, offset=0, ap=[[0, 128], [1, DOUT]])

    with tile.TileContext(nc) as tc, ExitStack() as ctx:
        sb = ctx.enter_context(tc.tile_pool(name="sb", bufs=1))

        # ---- resident weight/bias loads ----
        wq_sb = sb.tile([128, NK, DOUT], F16, tag="wq")
        wk_sb = sb.tile([128, NK, DOUT], F16, tag="wk")
        wv_sb = sb.tile([128, NK, DOUT], BF16, tag="wv")
        wo_sb = sb.tile([128, NPAIR, D], F16, tag="wo")
        bq_sb = sb.tile([128, NPAIR, 1], F32, tag="bq")
        bk_sb = sb.tile([128, NPAIR, 1], F32, tag="bk")
        bv_sb = sb.tile([128, DOUT], F32, tag="bv")

        # ---- resident activations ----
        qT_sb = sb.tile([128, NK, S], F16, tag="qT")
        kT_sb = sb.tile([128, NK, S], F16, tag="kT")
        vT_sb = sb.tile([128, NK, S], BF16, tag="vT")

        # head loads, ordered by first use; big tensors split into
        # token-slices so compute starts on partial data (DMA engines
        # serialize in practice)
        nc.sync.dma_start(out=bq_sb[:], in_=bqv)
        nc.sync.dma_start(out=wq_sb[:], in_=wqv)
        s0 = slice(0, 512)
        for kk in range(NK):  # q tokens 0:1024 -> proj chunks 0,1
            nc.sync.dma_start(out=qT_sb[:, kk, 0:1024], in_=qTv[kk][:, 0:1024])
        # k-path weights aren't needed until the k-proj chunk (~11us in)
        nc.sync.dma_start(out=wk_sb[:], in_=wkv)
        nc.sync.dma_start(out=bk_sb[:], in_=bkv)
        for kk in range(NK):  # k tokens 0:512 -> k-proj chunk 0
            nc.sync.dma_start(out=kT_sb[:, kk, s0], in_=kTv[kk][:, s0])
        for kk in range(NK):  # q tokens 1024:2048 -> proj chunks 2,3
            nc.sync.dma_start(out=qT_sb[:, kk, 1024:2048], in_=qTv[kk][:, 1024:2048])
        # v-path weights are not needed until v-proj (~20us in)
        nc.sync.dma_start(out=wv_sb[:], in_=wvv)
        nc.sync.dma_start(out=bv_sb[:], in_=bv_bcast_ap)

        # ---- projection outputs (resident SBUF) ----
        qpT_sb = sb.tile([128, NPAIR, S], F16, tag="qpT")
        kpT_sb = sb.tile([128, NPAIR, S], F16, tag="kpT")
        vp_sb = sb.tile([128, NTT, DOUT], BF16, tag="vp")
        hcT_sb = sb.tile([128, NPAIR, S], F16, tag="hcT")

        # PSUM: sc bufs=2 (4 banks, scores + all projections) +
        # pvt0/pvt1 resident PV accumulators (4 banks) = 8
        psa = ctx.enter_context(tc.tile_pool(name="ps_all", bufs=1, space="PSUM"))
        asb = ctx.enter_context(tc.tile_pool(name="att_sb", bufs=1))
        osb = ctx.enter_context(tc.tile_pool(name="o_sb", bufs=1))

        # warm the exp table during head DMA (table load ~2.7us)
        warm = sb.tile([128, 1], F32, tag="warm")
        nc.scalar.activation(out=warm[:], in_=bq_sb[:, 0, :], func=AF.Exp)

        def ps_tile(name):
            return psa.tile([128, 1024], F32, tag="sc", bufs=2, name=name)

        def emit_qkproj(X_sb, W_sb, b_sb, XPT, tci, p, tsl=None):
            # one 512-token chunk of a q/k projection for ONE pair
            if tsl is None:
                tsl = slice(tci * 512, tci * 512 + 512)
            n = tsl.stop - tsl.start
            ps_t = ps_tile(f"pj{p}")
            for kk in range(NK):
                nc.tensor.matmul(
                    ps_t[:, :n],
                    lhsT=W_sb[:, kk, p * 128 : p * 128 + 128],
                    rhs=X_sb[:, kk, tsl],
                    start=(kk == 0),
                    stop=(kk == NK - 1),
                )
            # bias-add gates the next scores' rhs: outrank the DVE z-chains
            with tc.high_priority():
                nc.vector.tensor_scalar_add(
                    XPT[:, p, tsl], ps_t[:, :n], b_sb[:, p, :]
                )

        def emit_vproj(tt):
            psv = ps_tile("projv")
            for kk in range(NK):
                nc.tensor.matmul(
                    psv[:, :DOUT],
                    lhsT=vT_sb[:, kk, tt * 128 : tt * 128 + 128],
                    rhs=wv_sb[:, kk, :],
                    start=(kk == 0),
                    stop=(kk == NK - 1),
                )
            nc.vector.scalar_tensor_tensor(
                out=vp_sb[:, tt, :],
                in0=psv[:, :DOUT],
                scalar=1.0,
                in1=bv_sb[:],
                op0=ALU.mult,
                op1=ALU.add,
            )

        def emit_scores(p, kb, qhs=None, scs=None):
            ksl = slice(kb * 128, kb * 128 + 128)
            if scs is None:
                scs = {}
            for qh in qhs if qhs is not None else range(NQH):
                for h in range(2):
                    sc = ps_tile(f"sc{h}{qh}")
                    for qq in range(2):
                        hsl = slice(h * 64, h * 64 + 64)
                        qsl = slice(
                            qh * 1024 + qq * 512, qh * 1024 + qq * 512 + 512
                        )
                        nc.tensor.matmul(
                            sc[:, qq * 512 : qq * 512 + 512],
                            lhsT=kpT_sb[hsl, p, ksl],
                            rhs=qpT_sb[hsl, p, qsl],
                            start=True,
                            stop=True,
                        )
                    scs[(h, qh)] = sc
            return scs

        def emit_exps(scs):
            # qh-major: the qh=0 exps only need the qh=0 score tiles, so
            # they clear the strict ACT FIFO before qh=1 scores are ready
            at_tiles, z_parts = {}, {}
            for qh in range(NQH):
                for h in range(2):
                    at = asb.tile(
                        [128, 1024], BF16, tag=f"at{h}_{qh}", bufs=5,
                        name=f"at{h}{qh}",
                    )
                    z = asb.tile(
                        [128, 1], F32, tag=f"z{h}_{qh}", bufs=5, name=f"z{h}{qh}"
                    )
                    nc.scalar.activation(
                        out=at[:], in_=scs[(h, qh)][:], func=AF.Exp,
                        accum_out=z[:],
                    )
                    at_tiles[(h, qh)] = at
                    z_parts[(h, qh)] = z
            return at_tiles, z_parts

        def emit_pv(p, kb, at_tiles, z_parts, pvts, hs=(0, 1)):
            # h=0 right after this kb's exps (its Z-chain gate is exp#3);
            # h=1 is software-pipelined one kb later: the PE is strictly
            # in-order, so a stalled pv-h1 would block everything behind it
            for h in hs:
                if NQH > 1:
                    zs = asb.tile([128, 1], F32, tag=f"zs{h}", bufs=2, name="zs")
                    nc.vector.tensor_add(
                        zs[:], z_parts[(h, 0)][:], z_parts[(h, 1)][:]
                    )
                    for qh in range(2, NQH):
                        nc.vector.tensor_add(zs[:], zs[:], z_parts[(h, qh)][:])
                else:
                    zs = z_parts[(h, 0)]
                rz = asb.tile([128, 1], F32, tag=f"rz{h}", bufs=2, name="rz")
                nc.vector.reciprocal(rz[:], zs[:])
                vhs = asb.tile([128, HD], BF16, tag=f"vh{h}", bufs=2, name="vhs")
                nc.vector.tensor_scalar_mul(
                    vhs[:],
                    vp_sb[:, kb, p * 128 + h * 64 : p * 128 + h * 64 + 64],
                    rz[:],
                )
                # accumulate in PSUM across all kb; one accumulation group
                # per bank (start only at kb==0 h==0, stop at the very end)
                for qh in range(NQH):
                    for qq in range(2):
                        nc.tensor.matmul(
                            pvts[qh][
                                h * 64 : h * 64 + 64,
                                qq * 512 : qq * 512 + 512,
                            ],
                            lhsT=vhs[:],
                            rhs=at_tiles[(h, qh)][
                                :, qq * 512 : qq * 512 + 512
                            ],
                            start=(kb == 0),
                            stop=(kb == NKB - 1),
                            tile_position=(0, h * 64),
                            skip_group_check=True,
                        )

        def emit_oproj(p, tt, tail=False):
            # one token tile of pair-p's O-projection partial
            ost = osb.tile([128, D], F16, tag="ost", bufs=7, name="ost")
            if tail:
                # pvt banks are drained by now: rotate through 3 PSUM tags
                # for a deeper pipeline
                tag = ("sc", "pvt0", "pvt1")[tt % 3]
                if tag == "sc":
                    ps_t = ps_tile(f"o{tt}")
                else:
                    ps_t = psa.tile([128, 1024], F32, tag=tag, bufs=1, name=f"o{tt}")
            else:
                ps_t = ps_tile(f"o{tt % 2}")
            for dc in range(2):
                nc.tensor.matmul(
                    ps_t[:, dc * 512 : dc * 512 + 512],
                    lhsT=hcT_sb[:, p, tt * 128 : tt * 128 + 128],
                    rhs=wo_sb[:, p, dc * 512 : dc * 512 + 512],
                    start=True,
                    stop=True,
                )
            if tail:
                # ACT is idle at the tail: split the copy across both engines
                nc.vector.tensor_copy(ost[:, 0:512], ps_t[:, 0:512])
                nc.scalar.copy(ost[:, 512:1024], ps_t[:, 512:1024])
            else:
                nc.vector.tensor_copy(ost[:], ps_t[:])
            nc.sync.dma_start(out=outv[p][tt], in_=ost[:])

        # ---- remaining DMA waves, ordered by consumption time ----
        for kk in range(NK):  # v tokens 0:512 -> v-proj tiles 0..3
            nc.sync.dma_start(out=vT_sb[:, kk, s0], in_=vTv[kk][:, s0])
        for sl in (slice(512, 1024), slice(1024, 1536), slice(1536, 2048)):
            for kk in range(NK):
                nc.sync.dma_start(out=kT_sb[:, kk, sl], in_=kTv[kk][:, sl])
            for kk in range(NK):
                nc.sync.dma_start(out=vT_sb[:, kk, sl], in_=vTv[kk][:, sl])
        nc.sync.dma_start(out=wo_sb[:], in_=wov)

        # ---- head, ordered to match DMA arrival: q chunks 0,1 then the
        # first k chunk, then the qh=0 scores of kb0 (so the first exps can
        # fire), then q chunks 2,3 and the qh=1 scores ----
        emit_qkproj(qT_sb, wq_sb, bq_sb, qpT_sb, 0, 0)
        emit_qkproj(qT_sb, wq_sb, bq_sb, qpT_sb, 1, 0)
        emit_qkproj(kT_sb, wk_sb, bk_sb, kpT_sb, 0, 0)
        scs0 = {}
        emit_scores(0, 0, qhs=[0], scs=scs0)
        emit_qkproj(qT_sb, wq_sb, bq_sb, qpT_sb, 2, 0)
        emit_qkproj(qT_sb, wq_sb, bq_sb, qpT_sb, 3, 0)
        emit_scores(0, 0, qhs=[1], scs=scs0)

        # remaining projection chunks: kb -> chunk.  qk chunks sit ALONE on
        # their kb (sc-rotation starves with vproj+qkproj on one kb); vproj
        # runs in pairs on the other kbs, always ahead of its PV deadline.
        proj_sched = {
            2: (kT_sb, wk_sb, bk_sb, kpT_sb, 1, 0),
            4: (kT_sb, wk_sb, bk_sb, kpT_sb, 2, 0),
            6: (kT_sb, wk_sb, bk_sb, kpT_sb, 3, 0),
            8: (qT_sb, wq_sb, bq_sb, qpT_sb, 0, 1),
            9: (qT_sb, wq_sb, bq_sb, qpT_sb, 1, 1),
            10: (qT_sb, wq_sb, bq_sb, qpT_sb, 2, 1),
            11: (qT_sb, wq_sb, bq_sb, qpT_sb, 3, 1),
            12: (kT_sb, wk_sb, bk_sb, kpT_sb, 0, 1),
        }
        # k-p1 chunks 1..3 are not needed until p1 kbs 4/8/12: emit them
        # inside the p1 phase to rebalance PE load between the two phases
        p1_sched = {
            1: (kT_sb, wk_sb, bk_sb, kpT_sb, 1, 1),
            5: (kT_sb, wk_sb, bk_sb, kpT_sb, 2, 1),
            9: (kT_sb, wk_sb, bk_sb, kpT_sb, 3, 1),
        }

        # resident PV accumulators (allocated once, reused across pairs)
        pvts = {
            qh: psa.tile([128, 1024], F32, tag=f"pvt{qh}", bufs=1, name=f"pvt{qh}")
            for qh in range(NQH)
        }

        def attention(p, per_kb_extra, scs=None):
            nonlocal pvts
            if scs is None:
                with tc.high_priority():
                    scs = emit_scores(p, 0)
            if p == 0:
                # after the first scores: emitting it earlier would push the
                # first exp's PE-counter wait past the vT DMA arrival
                emit_vproj(0)
            pending = None
            for kb in range(NKB):
                at_tiles, z_parts = emit_exps(scs)
                if pending is not None:
                    # previous kb's h=1 PV: its gate (exp4 + z-chain of the
                    # previous window) is long past, so it runs stall-free
                    emit_pv(p, kb - 1, *pending, pvts, hs=(1,))
                per_kb_extra(kb)
                if kb + 1 < NKB:
                    # allocation order stays extras-first (good WAR
                    # aliasing) but the scores matmuls get top scheduler
                    # priority: they feed the ACT exp stream, which ends
                    # the kernel — extras can always wait
                    with tc.high_priority():
                        scs = emit_scores(p, kb + 1)
                emit_pv(p, kb, at_tiles, z_parts, pvts, hs=(0,))
                pending = (at_tiles, z_parts)
            emit_pv(p, NKB - 1, *pending, pvts, hs=(1,))
            # drain PV accumulators -> fp16 O-proj lhsT; high priority:
            # these copies gate every O-projection tile of this pair
            with tc.high_priority():
                for qh in range(NQH):
                    pvt = pvts[qh]
                    qsl = slice(qh * 1024, qh * 1024 + 1024)
                    nc.vector.tensor_copy(hcT_sb[:, p, qsl], pvt[:])
            if p == 0:
                # re-allocate the same tags for pair 1 (WAR via pool deps)
                pvts = {
                    qh: psa.tile(
                        [128, 1024], F32, tag=f"pvt{qh}", bufs=1, name=f"pvt{qh}b"
                    )
                    for qh in range(NQH)
                }

        def p0_extra(kb):
            if kb + 1 < NKB:
                emit_vproj(kb + 1)
            if kb in proj_sched:
                emit_qkproj(*proj_sched.pop(kb))

        attention(0, p0_extra, scs=scs0)
        for kb in sorted(proj_sched):
            emit_qkproj(*proj_sched.pop(kb))

        def p1_extra(kb):
            # O-proj pairs on even kbs (empirically best of the tested
            # phase/parity layouts)
            if kb % 2 == 0:
                emit_oproj(0, kb)
                emit_oproj(0, kb + 1)
            if kb in p1_sched:
                emit_qkproj(*p1_sched.pop(kb))

        attention(1, p1_extra)

        # ---- tail: pair-1 O-projection ----
        for tt in range(NTT):
            emit_oproj(1, tt, tail=True)

    nc.compile()
    return nc


# ---------------- host-side shard / unshard ----------------

S = 2048
B = 2

_NC_CACHE = {}


def _get_nc():
    if "nc" not in _NC_CACHE:
        _NC_CACHE["nc"] = build_kernel(S=S)
    return _NC_CACHE["nc"]


def make_in_maps(q, k, v, Wq, bq, Wk, bk, Wv, bv, Wo, bo):
    bf = ml_dtypes.bfloat16
    f16 = np.float16
    maps = []
    for c in range(8):
        b = c // 4
        hc = c % 4
        cols = slice(256 * hc, 256 * hc + 256)
        maps.append({
            "qT": np.ascontiguousarray(q[b].T.astype(f16)),
            "kT": np.ascontiguousarray(k[b].T.astype(f16)),
            "vT": np.ascontiguousarray(v[b].astype(bf).T),
            "wq": np.ascontiguousarray(Wq[:, cols].astype(f16)),
            "wk": np.ascontiguousarray(Wk[:, cols].astype(f16)),
            "wv": np.ascontiguousarray(Wv[:, cols].astype(bf)),
            "wo": np.ascontiguousarray(Wo[cols, :].astype(f16)),
            "bq": np.ascontiguousarray(
                bq[cols].reshape(NPAIR, 128, 1).astype(np.float32)
            ),
            "bk": np.ascontiguousarray(
                bk[cols].reshape(NPAIR, 128, 1).astype(np.float32)
            ),
            "bv": np.ascontiguousarray(bv[cols].astype(np.float32)),
        })
    return maps


def kernel(q, k, v, Wq, bq, Wk, bk, Wv, bv, Wo, bo):
    q = np.asarray(q, dtype=np.float32)
    k = np.asarray(k, dtype=np.float32)
    v = np.asarray(v, dtype=np.float32)
    Wq = np.asarray(Wq, dtype=np.float32)
    Wk = np.asarray(Wk, dtype=np.float32)
    Wv = np.asarray(Wv, dtype=np.float32)
    Wo = np.asarray(Wo, dtype=np.float32)
    bq = np.asarray(bq, dtype=np.float32)
    bk = np.asarray(bk, dtype=np.float32)
    bv = np.asarray(bv, dtype=np.float32)
    bo = np.asarray(bo, dtype=np.float32)

    nc = _get_nc()
    maps = make_in_maps(q, k, v, Wq, bq, Wk, bk, Wv, bv, Wo, bo)
    res = run_bass_kernel_spmd(nc, maps, core_ids=list(range(8)))

    outs = []
    for b in range(B):
        acc = np.zeros((S, D), dtype=np.float32)
        for hc in range(4):
            r = res.results[b * 4 + hc]
            acc += r["out0"].astype(np.float32)
            acc += r["out1"].astype(np.float32)
        acc += bo[None, :]
        outs.append(acc)
    return np.stack(outs, axis=0)


# revision 87
# speedup vs baseline: 1.0020x; 1.0003x over previous
"""Bass/Tile kernel for nn_MultiHeadAttention (B=2, S=2048, D=1024, H=16).

Sharding: 8 cores = 2 (batch) x 4 (head-chunks of 4 heads).
Each core computes, for its batch b and its 4 heads (2 pairs of 2):
  qpT/kpT = (x @ W{q,k} + b)^T  in [dout, token] fp16 layout
  vp      = v @ Wv + bv         in [token, dout] bf16 layout
  scoresT = kp @ qp^T           per head, [k, q] fp32 PSUM
  attnT   = exp(scoresT)        (softmax over q == free axis) -> bf16
  Z[k]    = sum_q attnT[k, q]   (ACT accum_out, fp32)
  outT    = sum_kb (vp[kb]/Z[kb]) PV matmuls, accumulated IN PSUM
            across all 16 k-blocks (pvt0/pvt1 resident banks)
  out_p   = hcT_p^T @ Wo_p      per-pair fp16 partials (host sums 8)

PSUM: sc tag bufs=2 (4 banks, shared by scores AND all projection
chunks) + pvt0/pvt1 resident accumulators (4 banks) = 8.
Schedule: pair-0 q/k proj head; v-proj + remaining projections trickle
through pair-0 attention; pair-0's O-projection trickles through
pair-1 attention; tail is only pair-1's O-projection.
"""

import sys

sys.path.insert(0, "/opt/trn_rl_repo")

from contextlib import ExitStack

import numpy as np
import ml_dtypes

import concourse.bass as bass
import concourse.mybir as mybir
import concourse.tile as tile
from concourse import bacc
from concourse.bass_utils import run_bass_kernel_spmd

BF16 = mybir.dt.bfloat16
F16 = mybir.dt.float16
F32 = mybir.dt.float32
AF = mybir.ActivationFunctionType
ALU = mybir.AluOpType

D = 1024
NK = 8  # k-tiles over D
DOUT = 256  # per-core head dims (4 heads)
NPAIR = 2  # pairs of heads (128 dout each)
HD = 64


def build_kernel(S=2048):
    NKB = S // 128  # k-token blocks
    NQH = S // 1024  # exp blocks of 1024 along q
    NTC = S // 512  # proj token chunks
    NTT = S // 128  # token tiles
    assert S % 1024 == 0

    nc = bacc.Bacc("TRN2", target_bir_lowering=False, debug=False)

    qT = nc.dram_tensor("qT", [D, S], F16, kind="ExternalInput")
    kT = nc.dram_tensor("kT", [D, S], F16, kind="ExternalInput")
    vT = nc.dram_tensor("vT", [D, S], BF16, kind="ExternalInput")
    wq = nc.dram_tensor("wq", [D, DOUT], F16, kind="ExternalInput")
    wk = nc.dram_tensor("wk", [D, DOUT], F16, kind="ExternalInput")
    wv = nc.dram_tensor("wv", [D, DOUT], BF16, kind="ExternalInput")
    wo = nc.dram_tensor("wo", [DOUT, D], F16, kind="ExternalInput")
    bq = nc.dram_tensor("bq", [NPAIR, 128, 1], F32, kind="ExternalInput")
    bk = nc.dram_tensor("bk", [NPAIR, 128, 1], F32, kind="ExternalInput")
    bv = nc.dram_tensor("bv", [DOUT], F32, kind="ExternalInput")
    out0 = nc.dram_tensor("out0", [S, D], F16, kind="ExternalOutput")
    out1 = nc.dram_tensor("out1", [S, D], F16, kind="ExternalOutput")

    # tiled DRAM views
    qTv = qT.ap().rearrange("(t p) s -> t p s", p=128)  # [8, 128, S]
    kTv = kT.ap().rearrange("(t p) s -> t p s", p=128)
    vTv = vT.ap().rearrange("(t p) s -> t p s", p=128)
    wqv = wq.ap().rearrange("(t p) m -> p t m", p=128)  # [128, 8, 256]
    wkv = wk.ap().rearrange("(t p) m -> p t m", p=128)
    wvv = wv.ap().rearrange("(t p) m -> p t m", p=128)
    wov = wo.ap().rearrange("(t p) m -> p t m", p=128)  # [128, 2, 1024]
    bqv = bq.ap().rearrange("a p o -> p a o")  # [128, 2, 1]
    bkv = bk.ap().rearrange("a p o -> p a o")
    outv = [
        out0.ap().rearrange("(t p) m -> t p m", p=128),  # [NTT, 128, 1024]
        out1.ap().rearrange("(t p) m -> t p m", p=128),
    ]

    bv_bcast_ap = bass.AP(tensor=bv.ap().tensor, offset=0, ap=[[0, 128], [1, DOUT]])

    with tile.TileContext(nc) as tc, ExitStack() as ctx:
        sb = ctx.enter_context(tc.tile_pool(name="sb", bufs=1))

        # ---- resident weight/bias loads ----
        wq_sb = sb.tile([128, NK, DOUT], F16, tag="wq")
        wk_sb = sb.tile([128, NK, DOUT], F16, tag="wk")
        wv_sb = sb.tile([128, NK, DOUT], BF16, tag="wv")
        wo_sb = sb.tile([128, NPAIR, D], F16, tag="wo")
        bq_sb = sb.tile([128, NPAIR, 1], F32, tag="bq")
        bk_sb = sb.tile([128, NPAIR, 1], F32, tag="bk")
        bv_sb = sb.tile([128, DOUT], F32, tag="bv")

        # ---- resident activations ----
        qT_sb = sb.tile([128, NK, S], F16, tag="qT")
        kT_sb = sb.tile([128, NK, S], F16, tag="kT")
        vT_sb = sb.tile([128, NK, S], BF16, tag="vT")

        # head loads, ordered by first use; big tensors split into
        # token-slices so compute starts on partial data (DMA engines
        # serialize in practice)
        nc.sync.dma_start(out=bq_sb[:], in_=bqv)
        nc.sync.dma_start(out=wq_sb[:], in_=wqv)
        s0 = slice(0, 512)
        for kk in range(NK):  # q tokens 0:1024 -> proj chunks 0,1
            nc.sync.dma_start(out=qT_sb[:, kk, 0:1024], in_=qTv[kk][:, 0:1024])
        # k-path weights aren't needed until the k-proj chunk (~11us in)
        nc.sync.dma_start(out=wk_sb[:], in_=wkv)
        nc.sync.dma_start(out=bk_sb[:], in_=bkv)
        for kk in range(NK):  # k tokens 0:512 -> k-proj chunk 0
            nc.sync.dma_start(out=kT_sb[:, kk, s0], in_=kTv[kk][:, s0])
        for kk in range(NK):  # q tokens 1024:2048 -> proj chunks 2,3
            nc.sync.dma_start(out=qT_sb[:, kk, 1024:2048], in_=qTv[kk][:, 1024:2048])
        # v-path weights are not needed until v-proj (~20us in)
        nc.sync.dma_start(out=wv_sb[:], in_=wvv)
        nc.sync.dma_start(out=bv_sb[:], in_=bv_bcast_ap)

        # ---- projection outputs (resident SBUF) ----
        qpT_sb = sb.tile([128, NPAIR, S], F16, tag="qpT")
        kpT_sb = sb.tile([128, NPAIR, S], F16, tag="kpT")
        vp_sb = sb.tile([128, NTT, DOUT], BF16, tag="vp")
        hcT_sb = sb.tile([128, NPAIR, S], F16, tag="hcT")

        # PSUM: sc bufs=2 (4 banks, scores + all projections) +
        # pvt0/pvt1 resident PV accumulators (4 banks) = 8
        psa = ctx.enter_context(tc.tile_pool(name="ps_all", bufs=1, space="PSUM"))
        asb = ctx.enter_context(tc.tile_pool(name="att_sb", bufs=1))
        osb = ctx.enter_context(tc.tile_pool(name="o_sb", bufs=1))

        # warm the exp table during head DMA (table load ~2.7us)
        warm = sb.tile([128, 1], F32, tag="warm")
        nc.scalar.activation(out=warm[:], in_=bq_sb[:, 0, :], func=AF.Exp)

        def ps_tile(name):
            return psa.tile([128, 1024], F32, tag="sc", bufs=2, name=name)

        def emit_qkproj(X_sb, W_sb, b_sb, XPT, tci, p, tsl=None):
            # one 512-token chunk of a q/k projection for ONE pair
            if tsl is None:
                tsl = slice(tci * 512, tci * 512 + 512)
            n = tsl.stop - tsl.start
            ps_t = ps_tile(f"pj{p}")
            for kk in range(NK):
                nc.tensor.matmul(
                    ps_t[:, :n],
                    lhsT=W_sb[:, kk, p * 128 : p * 128 + 128],
                    rhs=X_sb[:, kk, tsl],
                    start=(kk == 0),
                    stop=(kk == NK - 1),
                )
            # bias-add gates the next scores' rhs: outrank the DVE z-chains
            with tc.high_priority():
                nc.vector.tensor_scalar_add(
                    XPT[:, p, tsl], ps_t[:, :n], b_sb[:, p, :]
                )

        def emit_vproj(tt):
            psv = ps_tile("projv")
            for kk in range(NK):
                nc.tensor.matmul(
                    psv[:, :DOUT],
                    lhsT=vT_sb[:, kk, tt * 128 : tt * 128 + 128],
                    rhs=wv_sb[:, kk, :],
                    start=(kk == 0),
                    stop=(kk == NK - 1),
                )
            nc.vector.scalar_tensor_tensor(
                out=vp_sb[:, tt, :],
                in0=psv[:, :DOUT],
                scalar=1.0,
                in1=bv_sb[:],
                op0=ALU.mult,
                op1=ALU.add,
            )

        def emit_scores(p, kb, qhs=None, scs=None):
            ksl = slice(kb * 128, kb * 128 + 128)
            if scs is None:
                scs = {}
            for qh in qhs if qhs is not None else range(NQH):
                for h in range(2):
                    sc = ps_tile(f"sc{h}{qh}")
                    for qq in range(2):
                        hsl = slice(h * 64, h * 64 + 64)
                        qsl = slice(
                            qh * 1024 + qq * 512, qh * 1024 + qq * 512 + 512
                        )
                        nc.tensor.matmul(
                            sc[:, qq * 512 : qq * 512 + 512],
                            lhsT=kpT_sb[hsl, p, ksl],
                            rhs=qpT_sb[hsl, p, qsl],
                            start=True,
                            stop=True,
                        )
                    scs[(h, qh)] = sc
            return scs

        def emit_exps(scs):
            # qh-major: the qh=0 exps only need the qh=0 score tiles, so
            # they clear the strict ACT FIFO before qh=1 scores are ready
            at_tiles, z_parts = {}, {}
            for qh in range(NQH):
                for h in range(2):
                    at = asb.tile(
                        [128, 1024], BF16, tag=f"at{h}_{qh}", bufs=5,
                        name=f"at{h}{qh}",
                    )
                    z = asb.tile(
                        [128, 1], F32, tag=f"z{h}_{qh}", bufs=5, name=f"z{h}{qh}"
                    )
                    nc.scalar.activation(
                        out=at[:], in_=scs[(h, qh)][:], func=AF.Exp,
                        accum_out=z[:],
                    )
                    at_tiles[(h, qh)] = at
                    z_parts[(h, qh)] = z
            return at_tiles, z_parts

        def emit_pv(p, kb, at_tiles, z_parts, pvts, hs=(0, 1)):
            # h=0 right after this kb's exps (its Z-chain gate is exp#3);
            # h=1 is software-pipelined one kb later: the PE is strictly
            # in-order, so a stalled pv-h1 would block everything behind it
            for h in hs:
                if NQH > 1:
                    zs = asb.tile([128, 1], F32, tag=f"zs{h}", bufs=3, name="zs")
                    nc.vector.tensor_add(
                        zs[:], z_parts[(h, 0)][:], z_parts[(h, 1)][:]
                    )
                    for qh in range(2, NQH):
                        nc.vector.tensor_add(zs[:], zs[:], z_parts[(h, qh)][:])
                else:
                    zs = z_parts[(h, 0)]
                rz = asb.tile([128, 1], F32, tag=f"rz{h}", bufs=3, name="rz")
                nc.vector.reciprocal(rz[:], zs[:])
                vhs = asb.tile([128, HD], BF16, tag=f"vh{h}", bufs=3, name="vhs")
                nc.vector.tensor_scalar_mul(
                    vhs[:],
                    vp_sb[:, kb, p * 128 + h * 64 : p * 128 + h * 64 + 64],
                    rz[:],
                )
                # accumulate in PSUM across all kb; one accumulation group
                # per bank (start only at kb==0 h==0, stop at the very end)
                for qh in range(NQH):
                    for qq in range(2):
                        nc.tensor.matmul(
                            pvts[qh][
                                h * 64 : h * 64 + 64,
                                qq * 512 : qq * 512 + 512,
                            ],
                            lhsT=vhs[:],
                            rhs=at_tiles[(h, qh)][
                                :, qq * 512 : qq * 512 + 512
                            ],
                            start=(kb == 0),
                            stop=(kb == NKB - 1),
                            tile_position=(0, h * 64),
                            skip_group_check=True,
                        )

        def emit_oproj(p, tt, tail=False):
            # one token tile of pair-p's O-projection partial
            ost = osb.tile([128, D], F16, tag="ost", bufs=8, name="ost")
            if tail:
                # pvt banks are drained by now: rotate through 3 PSUM tags
                # for a deeper pipeline
                tag = ("sc", "pvt0", "pvt1")[tt % 3]
                if tag == "sc":
                    ps_t = ps_tile(f"o{tt}")
                else:
                    ps_t = psa.tile([128, 1024], F32, tag=tag, bufs=1, name=f"o{tt}")
            else:
                ps_t = ps_tile(f"o{tt % 2}")
            for dc in range(2):
                nc.tensor.matmul(
                    ps_t[:, dc * 512 : dc * 512 + 512],
                    lhsT=hcT_sb[:, p, tt * 128 : tt * 128 + 128],
                    rhs=wo_sb[:, p, dc * 512 : dc * 512 + 512],
                    start=True,
                    stop=True,
                )
            if tail:
                # ACT is idle at the tail: split the copy across both engines
                nc.vector.tensor_copy(ost[:, 0:512], ps_t[:, 0:512])
                nc.scalar.copy(ost[:, 512:1024], ps_t[:, 512:1024])
            else:
                nc.vector.tensor_copy(ost[:], ps_t[:])
            nc.sync.dma_start(out=outv[p][tt], in_=ost[:])

        # ---- remaining DMA waves, ordered by consumption time ----
        for kk in range(NK):  # v tokens 0:512 -> v-proj tiles 0..3
            nc.sync.dma_start(out=vT_sb[:, kk, s0], in_=vTv[kk][:, s0])
        for sl in (slice(512, 1024), slice(1024, 1536), slice(1536, 2048)):
            for kk in range(NK):
                nc.sync.dma_start(out=kT_sb[:, kk, sl], in_=kTv[kk][:, sl])
            for kk in range(NK):
                nc.sync.dma_start(out=vT_sb[:, kk, sl], in_=vTv[kk][:, sl])
        nc.sync.dma_start(out=wo_sb[:], in_=wov)

        # ---- head, ordered to match DMA arrival: q chunks 0,1 then the
        # first k chunk, then the qh=0 scores of kb0 (so the first exps can
        # fire), then q chunks 2,3 and the qh=1 scores ----
        emit_qkproj(qT_sb, wq_sb, bq_sb, qpT_sb, 0, 0)
        emit_qkproj(qT_sb, wq_sb, bq_sb, qpT_sb, 1, 0)
        emit_qkproj(kT_sb, wk_sb, bk_sb, kpT_sb, 0, 0)
        scs0 = {}
        emit_scores(0, 0, qhs=[0], scs=scs0)
        emit_qkproj(qT_sb, wq_sb, bq_sb, qpT_sb, 2, 0)
        emit_qkproj(qT_sb, wq_sb, bq_sb, qpT_sb, 3, 0)
        emit_scores(0, 0, qhs=[1], scs=scs0)

        # remaining projection chunks: kb -> chunk.  qk chunks sit ALONE on
        # their kb (sc-rotation starves with vproj+qkproj on one kb); vproj
        # runs in pairs on the other kbs, always ahead of its PV deadline.
        proj_sched = {
            2: (kT_sb, wk_sb, bk_sb, kpT_sb, 1, 0),
            4: (kT_sb, wk_sb, bk_sb, kpT_sb, 2, 0),
            6: (kT_sb, wk_sb, bk_sb, kpT_sb, 3, 0),
            8: (qT_sb, wq_sb, bq_sb, qpT_sb, 0, 1),
            9: (qT_sb, wq_sb, bq_sb, qpT_sb, 1, 1),
            10: (qT_sb, wq_sb, bq_sb, qpT_sb, 2, 1),
            11: (qT_sb, wq_sb, bq_sb, qpT_sb, 3, 1),
            12: (kT_sb, wk_sb, bk_sb, kpT_sb, 0, 1),
        }
        # k-p1 chunks 1..3 are not needed until p1 kbs 4/8/12: emit them
        # inside the p1 phase to rebalance PE load between the two phases
        p1_sched = {
            1: (kT_sb, wk_sb, bk_sb, kpT_sb, 1, 1),
            5: (kT_sb, wk_sb, bk_sb, kpT_sb, 2, 1),
            9: (kT_sb, wk_sb, bk_sb, kpT_sb, 3, 1),
        }

        # resident PV accumulators (allocated once, reused across pairs)
        pvts = {
            qh: psa.tile([128, 1024], F32, tag=f"pvt{qh}", bufs=1, name=f"pvt{qh}")
            for qh in range(NQH)
        }

        def attention(p, per_kb_extra, scs=None):
            nonlocal pvts
            if scs is None:
                with tc.high_priority():
                    scs = emit_scores(p, 0)
            if p == 0:
                # after the first scores: emitting it earlier would push the
                # first exp's PE-counter wait past the vT DMA arrival
                emit_vproj(0)
            pending = None
            for kb in range(NKB):
                at_tiles, z_parts = emit_exps(scs)
                if pending is not None:
                    # previous kb's h=1 PV: its gate (exp4 + z-chain of the
                    # previous window) is long past, so it runs stall-free
                    emit_pv(p, kb - 1, *pending, pvts, hs=(1,))
                per_kb_extra(kb)
                if kb + 1 < NKB:
                    # allocation order stays extras-first (good WAR
                    # aliasing) but the scores matmuls get top scheduler
                    # priority: they feed the ACT exp stream, which ends
                    # the kernel — extras can always wait
                    with tc.high_priority():
                        scs = emit_scores(p, kb + 1)
                emit_pv(p, kb, at_tiles, z_parts, pvts, hs=(0,))
                pending = (at_tiles, z_parts)
            emit_pv(p, NKB - 1, *pending, pvts, hs=(1,))
            # drain PV accumulators -> fp16 O-proj lhsT; high priority:
            # these copies gate every O-projection tile of this pair
            with tc.high_priority():
                for qh in range(NQH):
                    pvt = pvts[qh]
                    qsl = slice(qh * 1024, qh * 1024 + 1024)
                    nc.vector.tensor_copy(hcT_sb[:, p, qsl], pvt[:])
            if p == 0:
                # re-allocate the same tags for pair 1 (WAR via pool deps)
                pvts = {
                    qh: psa.tile(
                        [128, 1024], F32, tag=f"pvt{qh}", bufs=1, name=f"pvt{qh}b"
                    )
                    for qh in range(NQH)
                }

        def p0_extra(kb):
            if kb + 1 < NKB:
                emit_vproj(kb + 1)
            if kb in proj_sched:
                emit_qkproj(*proj_sched.pop(kb))

        attention(0, p0_extra, scs=scs0)
        for kb in sorted(proj_sched):
            emit_qkproj(*proj_sched.pop(kb))

        def p1_extra(kb):
            # O-proj pairs on even kbs (empirically best of the tested
            # phase/parity layouts)
            if kb % 2 == 0:
                emit_oproj(0, kb)
                emit_oproj(0, kb + 1)
            if kb in p1_sched:
                emit_qkproj(*p1_sched.pop(kb))

        attention(1, p1_extra)

        # ---- tail: pair-1 O-projection ----
        for tt in range(NTT):
            emit_oproj(1, tt, tail=True)

    nc.compile()
    return nc


# ---------------- host-side shard / unshard ----------------

S = 2048
B = 2

_NC_CACHE = {}


def _get_nc():
    if "nc" not in _NC_CACHE:
        _NC_CACHE["nc"] = build_kernel(S=S)
    return _NC_CACHE["nc"]


def make_in_maps(q, k, v, Wq, bq, Wk, bk, Wv, bv, Wo, bo):
    bf = ml_dtypes.bfloat16
    f16 = np.float16
    maps = []
    for c in range(8):
        b = c // 4
        hc = c % 4
        cols = slice(256 * hc, 256 * hc + 256)
        maps.append({
            "qT": np.ascontiguousarray(q[b].T.astype(f16)),
            "kT": np.ascontiguousarray(k[b].T.astype(f16)),
            "vT": np.ascontiguousarray(v[b].astype(bf).T),
            "wq": np.ascontiguousarray(Wq[:, cols].astype(f16)),
            "wk": np.ascontiguousarray(Wk[:, cols].astype(f16)),
            "wv": np.ascontiguousarray(Wv[:, cols].astype(bf)),
            "wo": np.ascontiguousarray(Wo[cols, :].astype(f16)),
            "bq": np.ascontiguousarray(
                bq[cols].reshape(NPAIR, 128, 1).astype(np.float32)
            ),
            "bk": np.ascontiguousarray(
                bk[cols].reshape(NPAIR, 128, 1).astype(np.float32)
            ),
            "bv": np.ascontiguousarray(bv[cols].astype(np.float32)),
        })
    return maps


def kernel(q, k, v, Wq, bq, Wk, bk, Wv, bv, Wo, bo):
    q = np.asarray(q, dtype=np.float32)
    k = np.asarray(k, dtype=np.float32)
    v = np.asarray(v, dtype=np.float32)
    Wq = np.asarray(Wq, dtype=np.float32)
    Wk = np.asarray(Wk, dtype=np.float32)
    Wv = np.asarray(Wv, dtype=np.float32)
    Wo = np.asarray(Wo, dtype=np.float32)
    bq = np.asarray(bq, dtype=np.float32)
    bk = np.asarray(bk, dtype=np.float32)
    bv = np.asarray(bv, dtype=np.float32)
    bo = np.asarray(bo, dtype=np.float32)

    nc = _get_nc()
    maps = make_in_maps(q, k, v, Wq, bq, Wk, bk, Wv, bv, Wo, bo)
    res = run_bass_kernel_spmd(nc, maps, core_ids=list(range(8)))

    outs = []
    for b in range(B):
        acc = np.zeros((S, D), dtype=np.float32)
        for hc in range(4):
            r = res.results[b * 4 + hc]
            acc += r["out0"].astype(np.float32)
            acc += r["out1"].astype(np.float32)
        acc += bo[None, :]
        outs.append(acc)
    return np.stack(outs, axis=0)


# revision 90
# speedup vs baseline: 1.0024x; 1.0004x over previous
"""Bass/Tile kernel for nn_MultiHeadAttention (B=2, S=2048, D=1024, H=16).

Sharding: 8 cores = 2 (batch) x 4 (head-chunks of 4 heads).
Each core computes, for its batch b and its 4 heads (2 pairs of 2):
  qpT/kpT = (x @ W{q,k} + b)^T  in [dout, token] fp16 layout
  vp      = v @ Wv + bv         in [token, dout] bf16 layout
  scoresT = kp @ qp^T           per head, [k, q] fp32 PSUM
  attnT   = exp(scoresT)        (softmax over q == free axis) -> bf16
  Z[k]    = sum_q attnT[k, q]   (ACT accum_out, fp32)
  outT    = sum_kb (vp[kb]/Z[kb]) PV matmuls, accumulated IN PSUM
            across all 16 k-blocks (pvt0/pvt1 resident banks)
  out_p   = hcT_p^T @ Wo_p      per-pair fp16 partials (host sums 8)

PSUM: sc tag bufs=2 (4 banks, shared by scores AND all projection
chunks) + pvt0/pvt1 resident accumulators (4 banks) = 8.
Schedule: pair-0 q/k proj head; v-proj + remaining projections trickle
through pair-0 attention; pair-0's O-projection trickles through
pair-1 attention; tail is only pair-1's O-projection.
"""

import sys

sys.path.insert(0, "/opt/trn_rl_repo")

from contextlib import ExitStack

import numpy as np
import ml_dtypes

import concourse.bass as bass
import concourse.mybir as mybir
import concourse.tile as tile
from concourse import bacc
from concourse.bass_utils import run_bass_kernel_spmd

BF16 = mybir.dt.bfloat16
F16 = mybir.dt.float16
F32 = mybir.dt.float32
AF = mybir.ActivationFunctionType
ALU = mybir.AluOpType

D = 1024
NK = 8  # k-tiles over D
DOUT = 256  # per-core head dims (4 heads)
NPAIR = 2  # pairs of heads (128 dout each)
HD = 64


def build_kernel(S=2048):
    NKB = S // 128  # k-token blocks
    NQH = S // 1024  # exp blocks of 1024 along q
    NTC = S // 512  # proj token chunks
    NTT = S // 128  # token tiles
    assert S % 1024 == 0

    nc = bacc.Bacc("TRN2", target_bir_lowering=False, debug=False)

    qT = nc.dram_tensor("qT", [D, S], F16, kind="ExternalInput")
    kT = nc.dram_tensor("kT", [D, S], F16, kind="ExternalInput")
    vT = nc.dram_tensor("vT", [D, S], BF16, kind="ExternalInput")
    wq = nc.dram_tensor("wq", [D, DOUT], F16, kind="ExternalInput")
    wk = nc.dram_tensor("wk", [D, DOUT], F16, kind="ExternalInput")
    wv = nc.dram_tensor("wv", [D, DOUT], BF16, kind="ExternalInput")
    wo = nc.dram_tensor("wo", [DOUT, D], F16, kind="ExternalInput")
    bq = nc.dram_tensor("bq", [NPAIR, 128, 1], F32, kind="ExternalInput")
    bk = nc.dram_tensor("bk", [NPAIR, 128, 1], F32, kind="ExternalInput")
    bv = nc.dram_tensor("bv", [DOUT], F32, kind="ExternalInput")
    out0 = nc.dram_tensor("out0", [S, D], F16, kind="ExternalOutput")
    out1 = nc.dram_tensor("out1", [S, D], F16, kind="ExternalOutput")

    # tiled DRAM views
    qTv = qT.ap().rearrange("(t p) s -> t p s", p=128)  # [8, 128, S]
    kTv = kT.ap().rearrange("(t p) s -> t p s", p=128)
    vTv = vT.ap().rearrange("(t p) s -> t p s", p=128)
    wqv = wq.ap().rearrange("(t p) m -> p t m", p=128)  # [128, 8, 256]
    wkv = wk.ap().rearrange("(t p) m -> p t m", p=128)
    wvv = wv.ap().rearrange("(t p) m -> p t m", p=128)
    wov = wo.ap().rearrange("(t p) m -> p t m", p=128)  # [128, 2, 1024]
    bqv = bq.ap().rearrange("a p o -> p a o")  # [128, 2, 1]
    bkv = bk.ap().rearrange("a p o -> p a o")
    outv = [
        out0.ap().rearrange("(t p) m -> t p m", p=128),  # [NTT, 128, 1024]
        out1.ap().rearrange("(t p) m -> t p m", p=128),
    ]

    bv_bcast_ap = bass.AP(tensor=bv.ap().tensor, offset=0, ap=[[0, 128], [1, DOUT]])

    with tile.TileContext(nc) as tc, ExitStack() as ctx:
        sb = ctx.enter_context(tc.tile_pool(name="sb", bufs=1))

        # ---- resident weight/bias loads ----
        wq_sb = sb.tile([128, NK, DOUT], F16, tag="wq")
        wk_sb = sb.tile([128, NK, DOUT], F16, tag="wk")
        wv_sb = sb.tile([128, NK, DOUT], BF16, tag="wv")
        wo_sb = sb.tile([128, NPAIR, D], F16, tag="wo")
        bq_sb = sb.tile([128, NPAIR, 1], F32, tag="bq")
        bk_sb = sb.tile([128, NPAIR, 1], F32, tag="bk")
        bv_sb = sb.tile([128, DOUT], F32, tag="bv")

        # ---- resident activations ----
        qT_sb = sb.tile([128, NK, S], F16, tag="qT")
        kT_sb = sb.tile([128, NK, S], F16, tag="kT")
        vT_sb = sb.tile([128, NK, S], BF16, tag="vT")

        # head loads, ordered by first use; big tensors split into
        # token-slices so compute starts on partial data (DMA engines
        # serialize in practice)
        nc.sync.dma_start(out=bq_sb[:], in_=bqv)
        nc.sync.dma_start(out=wq_sb[:], in_=wqv)
        s0 = slice(0, 512)
        for kk in range(NK):  # q tokens 0:1024 -> proj chunks 0,1
            nc.sync.dma_start(out=qT_sb[:, kk, 0:1024], in_=qTv[kk][:, 0:1024])
        # k-path weights aren't needed until the k-proj chunk (~11us in)
        nc.sync.dma_start(out=wk_sb[:], in_=wkv)
        nc.sync.dma_start(out=bk_sb[:], in_=bkv)
        for kk in range(NK):  # k tokens 0:512 -> k-proj chunk 0
            nc.sync.dma_start(out=kT_sb[:, kk, s0], in_=kTv[kk][:, s0])
        for kk in range(NK):  # q tokens 1024:2048 -> proj chunks 2,3
            nc.sync.dma_start(out=qT_sb[:, kk, 1024:2048], in_=qTv[kk][:, 1024:2048])
        # v-path weights are not needed until v-proj (~20us in)
        nc.sync.dma_start(out=wv_sb[:], in_=wvv)
        nc.sync.dma_start(out=bv_sb[:], in_=bv_bcast_ap)

        # ---- projection outputs (resident SBUF) ----
        qpT_sb = sb.tile([128, NPAIR, S], F16, tag="qpT")
        kpT_sb = sb.tile([128, NPAIR, S], F16, tag="kpT")
        vp_sb = sb.tile([128, NTT, DOUT], BF16, tag="vp")
        hcT_sb = sb.tile([128, NPAIR, S], F16, tag="hcT")

        # PSUM: sc bufs=2 (4 banks, scores + all projections) +
        # pvt0/pvt1 resident PV accumulators (4 banks) = 8
        psa = ctx.enter_context(tc.tile_pool(name="ps_all", bufs=1, space="PSUM"))
        asb = ctx.enter_context(tc.tile_pool(name="att_sb", bufs=1))
        osb = ctx.enter_context(tc.tile_pool(name="o_sb", bufs=1))

        # warm the exp table during head DMA (table load ~2.7us)
        warm = sb.tile([128, 1], F32, tag="warm")
        nc.scalar.activation(out=warm[:], in_=bq_sb[:, 0, :], func=AF.Exp)

        def ps_tile(name):
            return psa.tile([128, 1024], F32, tag="sc", bufs=2, name=name)

        def emit_qkproj(X_sb, W_sb, b_sb, XPT, tci, p, tsl=None):
            # one 512-token chunk of a q/k projection for ONE pair
            if tsl is None:
                tsl = slice(tci * 512, tci * 512 + 512)
            n = tsl.stop - tsl.start
            ps_t = ps_tile(f"pj{p}")
            for kk in range(NK):
                nc.tensor.matmul(
                    ps_t[:, :n],
                    lhsT=W_sb[:, kk, p * 128 : p * 128 + 128],
                    rhs=X_sb[:, kk, tsl],
                    start=(kk == 0),
                    stop=(kk == NK - 1),
                )
            # bias-add gates the next scores' rhs: outrank the DVE z-chains
            with tc.high_priority():
                nc.vector.tensor_scalar_add(
                    XPT[:, p, tsl], ps_t[:, :n], b_sb[:, p, :]
                )

        def emit_vproj(tt):
            psv = ps_tile("projv")
            for kk in range(NK):
                nc.tensor.matmul(
                    psv[:, :DOUT],
                    lhsT=vT_sb[:, kk, tt * 128 : tt * 128 + 128],
                    rhs=wv_sb[:, kk, :],
                    start=(kk == 0),
                    stop=(kk == NK - 1),
                )
            nc.vector.scalar_tensor_tensor(
                out=vp_sb[:, tt, :],
                in0=psv[:, :DOUT],
                scalar=1.0,
                in1=bv_sb[:],
                op0=ALU.mult,
                op1=ALU.add,
            )

        def emit_scores(p, kb, qhs=None, scs=None):
            ksl = slice(kb * 128, kb * 128 + 128)
            if scs is None:
                scs = {}
            for qh in qhs if qhs is not None else range(NQH):
                for h in range(2):
                    sc = ps_tile(f"sc{h}{qh}")
                    for qq in range(2):
                        hsl = slice(h * 64, h * 64 + 64)
                        qsl = slice(
                            qh * 1024 + qq * 512, qh * 1024 + qq * 512 + 512
                        )
                        nc.tensor.matmul(
                            sc[:, qq * 512 : qq * 512 + 512],
                            lhsT=kpT_sb[hsl, p, ksl],
                            rhs=qpT_sb[hsl, p, qsl],
                            start=True,
                            stop=True,
                        )
                    scs[(h, qh)] = sc
            return scs

        def emit_exps(scs):
            # qh-major: the qh=0 exps only need the qh=0 score tiles, so
            # they clear the strict ACT FIFO before qh=1 scores are ready
            at_tiles, z_parts = {}, {}
            for qh in range(NQH):
                for h in range(2):
                    at = asb.tile(
                        [128, 1024], BF16, tag=f"at{h}_{qh}", bufs=5,
                        name=f"at{h}{qh}",
                    )
                    z = asb.tile(
                        [128, 1], F32, tag=f"z{h}_{qh}", bufs=5, name=f"z{h}{qh}"
                    )
                    nc.scalar.activation(
                        out=at[:], in_=scs[(h, qh)][:], func=AF.Exp,
                        accum_out=z[:],
                    )
                    at_tiles[(h, qh)] = at
                    z_parts[(h, qh)] = z
            return at_tiles, z_parts

        def emit_pv(p, kb, at_tiles, z_parts, pvts, hs=(0, 1)):
            # h=0 right after this kb's exps (its Z-chain gate is exp#3);
            # h=1 is software-pipelined one kb later: the PE is strictly
            # in-order, so a stalled pv-h1 would block everything behind it
            for h in hs:
                if NQH > 1:
                    zs = asb.tile([128, 1], F32, tag=f"zs{h}", bufs=3, name="zs")
                    nc.vector.tensor_add(
                        zs[:], z_parts[(h, 0)][:], z_parts[(h, 1)][:]
                    )
                    for qh in range(2, NQH):
                        nc.vector.tensor_add(zs[:], zs[:], z_parts[(h, qh)][:])
                else:
                    zs = z_parts[(h, 0)]
                rz = asb.tile([128, 1], F32, tag=f"rz{h}", bufs=3, name="rz")
                nc.vector.reciprocal(rz[:], zs[:])
                vhs = asb.tile([128, HD], BF16, tag=f"vh{h}", bufs=3, name="vhs")
                nc.vector.tensor_scalar_mul(
                    vhs[:],
                    vp_sb[:, kb, p * 128 + h * 64 : p * 128 + h * 64 + 64],
                    rz[:],
                )
                # accumulate in PSUM across all kb; one accumulation group
                # per bank (start only at kb==0 h==0, stop at the very end)
                for qh in range(NQH):
                    for qq in range(2):
                        nc.tensor.matmul(
                            pvts[qh][
                                h * 64 : h * 64 + 64,
                                qq * 512 : qq * 512 + 512,
                            ],
                            lhsT=vhs[:],
                            rhs=at_tiles[(h, qh)][
                                :, qq * 512 : qq * 512 + 512
                            ],
                            start=(kb == 0),
                            stop=(kb == NKB - 1),
                            tile_position=(0, h * 64),
                            skip_group_check=True,
                        )

        def emit_oproj(p, tt, tail=False):
            # one token tile of pair-p's O-projection partial
            ost = osb.tile([128, D], F16, tag="ost", bufs=9, name="ost")
            if tail:
                # pvt banks are drained by now: rotate through 3 PSUM tags
                # for a deeper pipeline
                tag = ("sc", "pvt0", "pvt1")[tt % 3]
                if tag == "sc":
                    ps_t = ps_tile(f"o{tt}")
                else:
                    ps_t = psa.tile([128, 1024], F32, tag=tag, bufs=1, name=f"o{tt}")
            else:
                ps_t = ps_tile(f"o{tt % 2}")
            for dc in range(2):
                nc.tensor.matmul(
                    ps_t[:, dc * 512 : dc * 512 + 512],
                    lhsT=hcT_sb[:, p, tt * 128 : tt * 128 + 128],
                    rhs=wo_sb[:, p, dc * 512 : dc * 512 + 512],
                    start=True,
                    stop=True,
                )
            if tail:
                # ACT is idle at the tail: split the copy across both engines
                nc.vector.tensor_copy(ost[:, 0:512], ps_t[:, 0:512])
                nc.scalar.copy(ost[:, 512:1024], ps_t[:, 512:1024])
            else:
                nc.vector.tensor_copy(ost[:], ps_t[:])
            nc.sync.dma_start(out=outv[p][tt], in_=ost[:])

        # ---- remaining DMA waves, ordered by consumption time ----
        for kk in range(NK):  # v tokens 0:512 -> v-proj tiles 0..3
            nc.sync.dma_start(out=vT_sb[:, kk, s0], in_=vTv[kk][:, s0])
        for sl in (slice(512, 1024), slice(1024, 1536), slice(1536, 2048)):
            for kk in range(NK):
                nc.sync.dma_start(out=kT_sb[:, kk, sl], in_=kTv[kk][:, sl])
            for kk in range(NK):
                nc.sync.dma_start(out=vT_sb[:, kk, sl], in_=vTv[kk][:, sl])
        nc.sync.dma_start(out=wo_sb[:], in_=wov)

        # ---- head, ordered to match DMA arrival: q chunks 0,1 then the
        # first k chunk, then the qh=0 scores of kb0 (so the first exps can
        # fire), then q chunks 2,3 and the qh=1 scores ----
        emit_qkproj(qT_sb, wq_sb, bq_sb, qpT_sb, 0, 0)
        emit_qkproj(qT_sb, wq_sb, bq_sb, qpT_sb, 1, 0)
        emit_qkproj(kT_sb, wk_sb, bk_sb, kpT_sb, 0, 0)
        scs0 = {}
        emit_scores(0, 0, qhs=[0], scs=scs0)
        emit_qkproj(qT_sb, wq_sb, bq_sb, qpT_sb, 2, 0)
        emit_qkproj(qT_sb, wq_sb, bq_sb, qpT_sb, 3, 0)
        emit_scores(0, 0, qhs=[1], scs=scs0)

        # remaining projection chunks: kb -> chunk.  qk chunks sit ALONE on
        # their kb (sc-rotation starves with vproj+qkproj on one kb); vproj
        # runs in pairs on the other kbs, always ahead of its PV deadline.
        proj_sched = {
            2: (kT_sb, wk_sb, bk_sb, kpT_sb, 1, 0),
            4: (kT_sb, wk_sb, bk_sb, kpT_sb, 2, 0),
            6: (kT_sb, wk_sb, bk_sb, kpT_sb, 3, 0),
            8: (qT_sb, wq_sb, bq_sb, qpT_sb, 0, 1),
            9: (qT_sb, wq_sb, bq_sb, qpT_sb, 1, 1),
            10: (qT_sb, wq_sb, bq_sb, qpT_sb, 2, 1),
            11: (qT_sb, wq_sb, bq_sb, qpT_sb, 3, 1),
            12: (kT_sb, wk_sb, bk_sb, kpT_sb, 0, 1),
        }
        # k-p1 chunks 1..3 are not needed until p1 kbs 4/8/12: emit them
        # inside the p1 phase to rebalance PE load between the two phases
        p1_sched = {
            1: (kT_sb, wk_sb, bk_sb, kpT_sb, 1, 1),
            5: (kT_sb, wk_sb, bk_sb, kpT_sb, 2, 1),
            9: (kT_sb, wk_sb, bk_sb, kpT_sb, 3, 1),
        }

        # resident PV accumulators (allocated once, reused across pairs)
        pvts = {
            qh: psa.tile([128, 1024], F32, tag=f"pvt{qh}", bufs=1, name=f"pvt{qh}")
            for qh in range(NQH)
        }

        def attention(p, per_kb_extra, scs=None):
            nonlocal pvts
            if scs is None:
                with tc.high_priority():
                    scs = emit_scores(p, 0)
            if p == 0:
                # after the first scores: emitting it earlier would push the
                # first exp's PE-counter wait past the vT DMA arrival
                emit_vproj(0)
            pending = None
            for kb in range(NKB):
                at_tiles, z_parts = emit_exps(scs)
                if pending is not None:
                    # previous kb's h=1 PV: its gate (exp4 + z-chain of the
                    # previous window) is long past, so it runs stall-free
                    emit_pv(p, kb - 1, *pending, pvts, hs=(1,))
                per_kb_extra(kb)
                if kb + 1 < NKB:
                    # allocation order stays extras-first (good WAR
                    # aliasing) but the scores matmuls get top scheduler
                    # priority: they feed the ACT exp stream, which ends
                    # the kernel — extras can always wait
                    with tc.high_priority():
                        scs = emit_scores(p, kb + 1)
                emit_pv(p, kb, at_tiles, z_parts, pvts, hs=(0,))
                pending = (at_tiles, z_parts)
            emit_pv(p, NKB - 1, *pending, pvts, hs=(1,))
            # drain PV accumulators -> fp16 O-proj lhsT; high priority:
            # these copies gate every O-projection tile of this pair
            with tc.high_priority():
                for qh in range(NQH):
                    pvt = pvts[qh]
                    qsl = slice(qh * 1024, qh * 1024 + 1024)
                    nc.vector.tensor_copy(hcT_sb[:, p, qsl], pvt[:])
            if p == 0:
                # re-allocate the same tags for pair 1 (WAR via pool deps)
                pvts = {
                    qh: psa.tile(
                        [128, 1024], F32, tag=f"pvt{qh}", bufs=1, name=f"pvt{qh}b"
                    )
                    for qh in range(NQH)
                }

        def p0_extra(kb):
            if kb + 1 < NKB:
                emit_vproj(kb + 1)
            if kb in proj_sched:
                emit_qkproj(*proj_sched.pop(kb))

        attention(0, p0_extra, scs=scs0)
        for kb in sorted(proj_sched):
            emit_qkproj(*proj_sched.pop(kb))

        def p1_extra(kb):
            # O-proj pairs on even kbs (empirically best of the tested
            # phase/parity layouts)
            if kb % 2 == 0:
                emit_oproj(0, kb)
                emit_oproj(0, kb + 1)
            if kb in p1_sched:
                emit_qkproj(*p1_sched.pop(kb))

        attention(1, p1_extra)

        # ---- tail: pair-1 O-projection ----
        for tt in range(NTT):
            emit_oproj(1, tt, tail=True)

    nc.compile()
    return nc


# ---------------- host-side shard / unshard ----------------

S = 2048
B = 2

_NC_CACHE = {}


def _get_nc():
    if "nc" not in _NC_CACHE:
        _NC_CACHE["nc"] = build_kernel(S=S)
    return _NC_CACHE["nc"]


def make_in_maps(q, k, v, Wq, bq, Wk, bk, Wv, bv, Wo, bo):
    bf = ml_dtypes.bfloat16
    f16 = np.float16
    maps = []
    for c in range(8):
        b = c // 4
        hc = c % 4
        cols = slice(256 * hc, 256 * hc + 256)
        maps.append({
            "qT": np.ascontiguousarray(q[b].T.astype(f16)),
            "kT": np.ascontiguousarray(k[b].T.astype(f16)),
            "vT": np.ascontiguousarray(v[b].astype(bf).T),
            "wq": np.ascontiguousarray(Wq[:, cols].astype(f16)),
            "wk": np.ascontiguousarray(Wk[:, cols].astype(f16)),
            "wv": np.ascontiguousarray(Wv[:, cols].astype(bf)),
            "wo": np.ascontiguousarray(Wo[cols, :].astype(f16)),
            "bq": np.ascontiguousarray(
                bq[cols].reshape(NPAIR, 128, 1).astype(np.float32)
            ),
            "bk": np.ascontiguousarray(
                bk[cols].reshape(NPAIR, 128, 1).astype(np.float32)
            ),
            "bv": np.ascontiguousarray(bv[cols].astype(np.float32)),
        })
    return maps


def kernel(q, k, v, Wq, bq, Wk, bk, Wv, bv, Wo, bo):
    q = np.asarray(q, dtype=np.float32)
    k = np.asarray(k, dtype=np.float32)
    v = np.asarray(v, dtype=np.float32)
    Wq = np.asarray(Wq, dtype=np.float32)
    Wk = np.asarray(Wk, dtype=np.float32)
    Wv = np.asarray(Wv, dtype=np.float32)
    Wo = np.asarray(Wo, dtype=np.float32)
    bq = np.asarray(bq, dtype=np.float32)
    bk = np.asarray(bk, dtype=np.float32)
    bv = np.asarray(bv, dtype=np.float32)
    bo = np.asarray(bo, dtype=np.float32)

    nc = _get_nc()
    maps = make_in_maps(q, k, v, Wq, bq, Wk, bk, Wv, bv, Wo, bo)
    res = run_bass_kernel_spmd(nc, maps, core_ids=list(range(8)))

    outs = []
    for b in range(B):
        acc = np.zeros((S, D), dtype=np.float32)
        for hc in range(4):
            r = res.results[b * 4 + hc]
            acc += r["out0"].astype(np.float32)
            acc += r["out1"].astype(np.float32)
        acc += bo[None, :]
        outs.append(acc)
    return np.stack(outs, axis=0)
